# revision 1
# baseline (speedup 1.0000x reference)
"""FAConv + LayerNorm + ReLU fused Trainium2 kernel (8 NeuronCores, SPMD).

Strategy:
  Host: sort edges by destination 128-node block (core k owns 49 blocks =
  a contiguous 6272-node output shard -> no all-reduce), split each block's
  edges by src < 25088 (int16 gather-index limit), pad per (block, half) to
  tiles of 128 edges.
  Phase A (data-parallel): per-core node shard -> a_l/a_r = node @ att_{l,r}
  (DVE mult + ScalarE accumulate), emit bf16 node table (512B rows) and
  per-node a_l/a_r scalars.
  Host: concat shards; permute a_l by edge src and a_r by edge dst into the
  padded tile layout (data movement only - all arithmetic stays on device).
  Phase B (edge-parallel): coef = tanh(a_l[src]+a_r[dst])*w computed as two
  whole-array DVE ops + one ScalarE tanh; per dst block, dma_gather node
  rows of edge sources (4 SWDGE queues in parallel); per 128-edge tile ONE
  DVE op builds the coef-scaled one-hot (iota==dst_local)*coef, segment-sum
  as PSUM-accumulated matmuls; fused +eps*node_0 -> LayerNorm -> ReLU
  epilogue per block.
"""
import sys

for _p in ('/opt/trn_rl_repo', '/root/.axon_site/_ro/trn_rl_repo'):
    if _p not in sys.path:
        sys.path.insert(0, _p)

import numpy as np
import ml_dtypes

import concourse.bass as bass
import concourse.bacc as bacc
import concourse.tile as tile
from concourse import mybir
from concourse.bass_utils import run_bass_kernel_spmd

N = 50000
D = 256
NCORES = 8
BPC = 49                    # dst blocks per core
NPAD = NCORES * BPC * 128   # 50176
NSH = BPC * 128             # 6272 nodes per core shard
HALF = NPAD // 2            # 25088 (int16-safe gather index range)
EPS_FA = 0.1
EPS_LN = 1e-5
MAXG = 8                    # max tiles (of 128 idxs) per dma_gather (ring cap 1024)

f32 = mybir.dt.float32
bf16 = mybir.dt.bfloat16
i16 = mybir.dt.int16
AF = mybir.ActivationFunctionType
OP = mybir.AluOpType

_cache = {}


def _build_phase_a():
    nc = bacc.Bacc("TRN2", target_bir_lowering=False, debug=False,
                   num_devices=NCORES)
    node_sh = nc.declare_dram_parameter("node_sh", [NSH, D], f32, isOutput=False)
    att = nc.declare_dram_parameter("att", [2, D], f32, isOutput=False)
    aug_sh = nc.declare_dram_parameter("aug_sh", [NSH, D], bf16, isOutput=True)
    alr_sh = nc.declare_dram_parameter("alr_sh", [BPC, 128, 2], f32, isOutput=True)

    with tile.TileContext(nc) as tc:
        with (
            tc.tile_pool(name="const", bufs=1) as cpool,
            tc.tile_pool(name="sbuf", bufs=8) as pool,
            tc.tile_pool(name="psum", bufs=2, space="PSUM") as psum,
        ):
            ones = cpool.tile([1, 128], f32)
            nc.vector.memset(ones[:], 1.0)
            att_bc = []
            for j in range(2):
                att_row = cpool.tile([1, D], f32, tag=f"attrow{j}")
                nc.sync.dma_start(out=att_row[:], in_=att[j:j + 1, :])
                ps = psum.tile([128, D], f32, tag="attps")
                nc.tensor.matmul(out=ps[:], lhsT=ones[:], rhs=att_row[:],
                                 start=True, stop=True)
                bc = cpool.tile([128, D], f32, tag=f"attbc{j}")
                nc.vector.tensor_copy(bc[:], ps[:])
                att_bc.append(bc)

            for i in range(BPC):
                nt = pool.tile([128, D], f32, tag="nt")
                nc.sync.dma_start(out=nt[:], in_=node_sh[i * 128:(i + 1) * 128, :])
                alr_t = pool.tile([128, 2], f32, tag="alr")
                scr = pool.tile([128, D], f32, tag="scr")
                nc.vector.tensor_tensor(out=scr[:], in0=nt[:], in1=att_bc[0][:],
                                        op=OP.mult)
                scrc = pool.tile([128, D], f32, tag="scrc")
                nc.scalar.activation(out=scrc[:], in_=scr[:], func=AF.Copy,
                                     accum_out=alr_t[:, 0:1])
                scr2 = pool.tile([128, D], f32, tag="scr2")
                nc.vector.tensor_tensor(out=scr2[:], in0=nt[:], in1=att_bc[1][:],
                                        op=OP.mult)
                scr2c = pool.tile([128, D], f32, tag="scr2c")
                nc.scalar.activation(out=scr2c[:], in_=scr2[:], func=AF.Copy,
                                     accum_out=alr_t[:, 1:2])
                aug_t = pool.tile([128, D], bf16, tag="aug")
                nc.scalar.activation(out=aug_t[:], in_=nt[:], func=AF.Copy)
                nc.sync.dma_start(out=aug_sh[i * 128:(i + 1) * 128, :], in_=aug_t[:])
                nc.sync.dma_start(out=alr_sh[i, :, :], in_=alr_t[:])
    nc.finalize()
    return nc


def _build_phase_b(t_lo, t_hi, gb_identity):
    TT = int(sum(t_lo) + sum(t_hi))          # total edge tiles
    SL = int(8 * sum(t_lo))                  # idx cols for lo stream
    SH = int(8 * sum(t_hi))
    nc = bacc.Bacc("TRN2", target_bir_lowering=False, debug=False,
                   num_devices=NCORES, num_swdge_queues=4)
    aug = nc.declare_dram_parameter("aug", [NPAD, D], bf16, isOutput=False)
    idx_lo = nc.declare_dram_parameter("idx_lo", [128, max(SL, 8)], i16, isOutput=False)
    idx_hi = nc.declare_dram_parameter("idx_hi", [128, max(SH, 8)], i16, isOutput=False)
    dstl = nc.declare_dram_parameter("dstl", [128, TT], f32, isOutput=False)
    wgt = nc.declare_dram_parameter("wgt", [128, TT], f32, isOutput=False)
    alv = nc.declare_dram_parameter("alv", [128, TT], f32, isOutput=False)
    arv = nc.declare_dram_parameter("arv", [128, TT], f32, isOutput=False)
    node0_sh = nc.declare_dram_parameter("node0_sh", [NSH, D], f32, isOutput=False)
    gb = nc.declare_dram_parameter("gb", [1, 2 * D], f32, isOutput=False)
    iota_in = nc.declare_dram_parameter("iota_in", [128, 128], bf16, isOutput=False)
    out_sh = nc.declare_dram_parameter("out_sh", [NSH, D], f32, isOutput=True)

    with tile.TileContext(nc) as tc:
        with (
            tc.tile_pool(name="const", bufs=1) as cpool,
            tc.tile_pool(name="gpool", bufs=24) as gpool,
            tc.tile_pool(name="work", bufs=8) as work,
            tc.tile_pool(name="epi", bufs=2) as epi,
            tc.tile_pool(name="psum", bufs=2, space="PSUM") as psum,
            tc.tile_pool(name="arpsum", bufs=1, space="PSUM") as arpsum,
        ):
            # constants
            iota_bf = cpool.tile([128, 128], bf16)
            nc.sync.dma_start(out=iota_bf[:], in_=iota_in[:, :])
            ones_f = cpool.tile([1, 128], f32)
            nc.vector.memset(ones_f[:], 1.0)
            gb_row = cpool.tile([1, 2 * D], f32)
            nc.sync.dma_start(out=gb_row[:], in_=gb[:, :])
            gb_ps = arpsum.tile([128, 2 * D], f32, tag="gbps")
            nc.tensor.matmul(out=gb_ps[:], lhsT=ones_f[:], rhs=gb_row[:],
                             start=True, stop=True)
            gb_bc = cpool.tile([128, 2 * D], f32)
            nc.vector.tensor_copy(gb_bc[:], gb_ps[:])

            # preload idx/dstl/w/al/ar streams
            ilo = cpool.tile([128, max(SL, 8)], i16, tag="ilo")
            nc.sync.dma_start(out=ilo[:], in_=idx_lo[:, :])
            ihi = cpool.tile([128, max(SH, 8)], i16, tag="ihi")
            nc.sync.dma_start(out=ihi[:], in_=idx_hi[:, :])
            dstl_sb = cpool.tile([128, TT], f32, tag="dstl")
            nc.sync.dma_start(out=dstl_sb[:], in_=dstl[:, :])
            w_sb = cpool.tile([128, TT], f32, tag="w")
            nc.sync.dma_start(out=w_sb[:], in_=wgt[:, :])
            al_sb = cpool.tile([128, TT], f32, tag="al")
            nc.sync.dma_start(out=al_sb[:], in_=alv[:, :])
            ar_sb = cpool.tile([128, TT], f32, tag="ar")
            nc.sync.dma_start(out=ar_sb[:], in_=arv[:, :])

            # whole-array coef = tanh(al + ar) * w   (3 ops total)
            arg_sb = cpool.tile([128, TT], f32, tag="arg")
            nc.vector.tensor_tensor(out=arg_sb[:], in0=al_sb[:], in1=ar_sb[:],
                                    op=OP.add)
            th_sb = cpool.tile([128, TT], f32, tag="th")
            nc.scalar.activation(out=th_sb[:], in_=arg_sb[:], func=AF.Tanh)
            coef_sb = cpool.tile([128, TT], f32, tag="coef")
            nc.vector.tensor_tensor(out=coef_sb[:], in0=th_sb[:], in1=w_sb[:],
                                    op=OP.mult)

            qctr = 0
            gt = 0          # global tile index (stream column)
            icol = {"lo": 0, "hi": 0}
            for i in range(BPC):
                n0 = epi.tile([128, D], f32, tag="n0")
                nc.sync.dma_start(out=n0[:], in_=node0_sh[i * 128:(i + 1) * 128, :])

                ti = int(t_lo[i] + t_hi[i])
                acc = psum.tile([128, D], f32, tag="acc")
                ts = 0
                for half, tcnt, istream, base in (
                        ("lo", int(t_lo[i]), ilo, aug[0:HALF, :]),
                        ("hi", int(t_hi[i]), ihi, aug[HALF:NPAD, :])):
                    done = 0
                    while done < tcnt:
                        c = min(MAXG, tcnt - done)
                        g = gpool.tile([128, MAXG, D], bf16, tag="g")
                        ic = icol[half]
                        nc.gpsimd.dma_gather(
                            out_ap=g[:, 0:c, :], in_ap=base,
                            idxs_ap=istream[:, ic:ic + 8 * c],
                            num_idxs=c * 128, num_idxs_reg=c * 128,
                            elem_size=D, queue_num=qctr % 4)
                        qctr += 1
                        icol[half] = ic + 8 * c
                        for tt in range(c):
                            stat = work.tile([128, 128], bf16, tag="stat")
                            nc.vector.tensor_scalar(
                                out=stat[:], in0=iota_bf[:],
                                scalar1=dstl_sb[:, gt:gt + 1],
                                scalar2=coef_sb[:, gt:gt + 1],
                                op0=OP.is_equal, op1=OP.mult)
                            nc.tensor.matmul(out=acc[:], lhsT=stat[:],
                                             rhs=g[:, tt, 0:D],
                                             start=(ts == 0), stop=(ts == ti - 1))
                            ts += 1
                            gt += 1
                        done += c

                # epilogue: x = acc + EPS_FA*node0 ; LayerNorm ; ReLU
                xe = epi.tile([128, D], f32, tag="xe")
                nc.scalar.activation(out=xe[:], in_=n0[:], func=AF.Copy,
                                     scale=EPS_FA)
                x = epi.tile([128, D], f32, tag="x")
                nc.vector.tensor_tensor(out=x[:], in0=xe[:], in1=acc[:], op=OP.add)
                sum_x = epi.tile([128, 1], f32, tag="sumx")
                xc = epi.tile([128, D], f32, tag="xc")
                nc.scalar.activation(out=xc[:], in_=x[:], func=AF.Copy,
                                     accum_out=sum_x[:])
                sumsq = epi.tile([128, 1], f32, tag="sumsq")
                xsq = epi.tile([128, D], f32, tag="xsq")
                nc.scalar.activation(out=xsq[:], in_=x[:], func=AF.Square,
                                     accum_out=sumsq[:])
                negmean = epi.tile([128, 1], f32, tag="negmean")
                nc.scalar.activation(out=negmean[:], in_=sum_x[:], func=AF.Copy,
                                     scale=-1.0 / D)
                msq = epi.tile([128, 1], f32, tag="msq")
                nc.scalar.activation(out=msq[:], in_=negmean[:], func=AF.Square)
                var = epi.tile([128, 1], f32, tag="var")
                nc.scalar.activation(out=var[:], in_=sumsq[:], func=AF.Copy,
                                     scale=1.0 / D, bias=EPS_LN)
                nc.vector.tensor_tensor(out=var[:], in0=var[:], in1=msq[:],
                                        op=OP.subtract)
                std = epi.tile([128, 1], f32, tag="std")
                nc.scalar.activation(out=std[:], in_=var[:], func=AF.Sqrt)
                rstd = epi.tile([128, 1], f32, tag="rstd")
                nc.vector.reciprocal(rstd[:], std[:])
                xn = epi.tile([128, D], f32, tag="xn")
                nc.vector.tensor_scalar(out=xn[:], in0=x[:], scalar1=negmean[:],
                                        scalar2=rstd[:], op0=OP.add, op1=OP.mult)
                if gb_identity:
                    y = xn
                else:
                    y = epi.tile([128, D], f32, tag="y")
                    nc.vector.tensor_tensor(out=y[:], in0=xn[:], in1=gb_bc[:, 0:D],
                                            op=OP.mult)
                    nc.vector.tensor_tensor(out=y[:], in0=y[:], in1=gb_bc[:, D:2 * D],
                                            op=OP.add)
                yr = epi.tile([128, D], f32, tag="yr")
                nc.scalar.activation(out=yr[:], in_=y[:], func=AF.Relu)
                nc.sync.dma_start(out=out_sh[i * 128:(i + 1) * 128, :], in_=yr[:])
    nc.finalize()
    return nc


def _pack_gather_idxs(stream_vals, t_caps, full_flags):
    """stream_vals: per-slot arrays of valid idxs (< 32768); t_caps: tiles per
    slot; full_flags[(slot, chunk)] True -> no -1 at all (first-touch slots).
    Packed per dma_gather call (chunks of <= MAXG tiles), 16-wrapped and
    replicated across the 8 Q7-core partition groups. Trailing -1 never
    swallows a full 128-idx tile (ucode crash)."""
    total_cols = 8 * int(sum(t_caps))
    arr = np.full((16, max(total_cols, 8)), -1, np.int16)
    col = 0
    for si, (vals, tcap) in enumerate(zip(stream_vals, t_caps)):
        tcap = int(tcap)
        done = 0
        ci = 0
        v = np.asarray(vals, np.int16)
        nv = len(v)
        while done < tcap:
            c = min(MAXG, tcap - done)
            # pads gather dummy row 0 (w=0 zeroes their contribution);
            # -1 skips are avoided entirely: a skipped row leaves a stale
            # partition that may be uninitialized (NaN) SBUF.
            chunk = np.zeros(c * 128, np.int16)
            lo = done * 128
            take = max(0, min(nv - lo, c * 128))
            if take:
                chunk[:take] = v[lo:lo + take]
            arr[:, col:col + 8 * c] = chunk.reshape(8 * c, 16).T
            col += 8 * c
            done += c
            ci += 1
    return np.tile(arr, (8, 1))


def kernel(node, node_0, edge_index, edge_attr, batch_ptr,
           att_l, att_r, ln_weight, ln_bias):
    node = np.asarray(node, np.float32)
    node_0 = np.asarray(node_0, np.float32)
    src = np.asarray(edge_index[0], np.int64)
    dst = np.asarray(edge_index[1], np.int64)
    w = np.asarray(edge_attr, np.float32)
    att_l = np.asarray(att_l, np.float32)
    att_r = np.asarray(att_r, np.float32)
    ln_weight = np.asarray(ln_weight, np.float32)
    ln_bias = np.asarray(ln_bias, np.float32)

    # ---- host sharding prep ----
    # load-balance: rank dst blocks by edge count; slot i of the 8 cores
    # holds the blocks ranked [8i, 8i+8) -> per-slot max ~= mean -> minimal
    # SPMD padding. Output rows are re-assembled per assignment at the end.
    blk = dst >> 7
    NB = NCORES * BPC
    bcnt = np.bincount(blk, minlength=NB)
    ranked = np.argsort(-bcnt, kind="stable")
    block2core = np.empty(NB, np.int64)
    block2slot = np.empty(NB, np.int64)
    for r, b in enumerate(ranked):
        block2core[b] = r % NCORES
        block2slot[b] = r // NCORES
    key = (block2core[blk] * BPC + block2slot[blk]) * 2 + (src >= HALF)
    order = np.argsort(key, kind="stable")
    src_s = src[order].astype(np.int32)
    dst_s = dst[order].astype(np.int32)
    dstl_s = (dst_s & 127).astype(np.float32)
    w_s = w[order]
    cnt = np.bincount(key[order], minlength=2 * NCORES * BPC)
    offs = np.concatenate([[0], np.cumsum(cnt)])
    cnt = cnt.reshape(NCORES, BPC, 2)
    t_lo = np.maximum(1, -(-cnt[:, :, 0].max(axis=0) // 128))   # [BPC]
    t_hi = np.maximum(1, -(-cnt[:, :, 1].max(axis=0) // 128))

    gb_identity = bool(np.all(ln_weight == 1.0) and np.all(ln_bias == 0.0))
    sig = (tuple(t_lo), tuple(t_hi), gb_identity)
    if "A" not in _cache:
        _cache["A"] = _build_phase_a()
    if ("B", sig) not in _cache:
        _cache[("B", sig)] = _build_phase_b(t_lo, t_hi, sig[2])
    nc_a = _cache["A"]
    nc_b = _cache[("B", sig)]

    # global gather-call order -> first-16 calls must have no -1 (uninit slots)
    flags_lo, flags_hi = {}, {}
    gidx = 0
    for i in range(BPC):
        for half, tcap, flags in ((0, int(t_lo[i]), flags_lo),
                                  (1, int(t_hi[i]), flags_hi)):
            nch = -(-tcap // MAXG)
            for ci in range(nch):
                if gidx < 24:
                    flags[(i, ci)] = True
                gidx += 1

    # ---- phase A ----
    node_pad = np.zeros((NPAD, D), np.float32)
    node_pad[:N] = node
    att = np.stack([att_l, att_r])
    in_a = [{"node_sh": node_pad[k * NSH:(k + 1) * NSH], "att": att}
            for k in range(NCORES)]
    res_a = run_bass_kernel_spmd(nc_a, in_a, list(range(NCORES)),
                                 **_cache.get("runkw", {}))
    aug_full = np.concatenate([res_a.results[k]["aug_sh"] for k in range(NCORES)])
    alr_full = np.concatenate(
        [res_a.results[k]["alr_sh"].reshape(NSH, 2) for k in range(NCORES)])
    al_full = np.ascontiguousarray(alr_full[:, 0])
    ar_full = np.ascontiguousarray(alr_full[:, 1])
    t_a = res_a.exec_time_ns

    # ---- phase B ----
    TT = int(t_lo.sum() + t_hi.sum())
    node0_pad = np.zeros((NPAD, D), np.float32)
    node0_pad[:N] = node_0
    gb = np.concatenate([ln_weight, ln_bias])[None, :]
    iota_np = np.tile(np.arange(128, dtype=np.float32).astype(
        ml_dtypes.bfloat16)[None, :], (128, 1))
    in_b = []
    for k in range(NCORES):
        lo_vals, hi_vals = [], []
        for i in range(BPC):
            for h, coll in ((0, lo_vals), (1, hi_vals)):
                ki = (2 * (k * BPC + i)) + h
                s0, s1 = offs[ki], offs[ki + 1]
                v = src_s[s0:s1]
                coll.append(v - HALF if h else v)
        dstl_arr = np.zeros((128, TT), np.float32)
        w_arr = np.zeros((128, TT), np.float32)
        al_arr = np.zeros((128, TT), np.float32)
        ar_arr = np.zeros((128, TT), np.float32)
        col = 0
        for i in range(BPC):
            for h, tcap in ((0, t_lo[i]), (1, t_hi[i])):
                ki = (2 * (k * BPC + i)) + h
                s0, s1 = offs[ki], offs[ki + 1]
                nv = s1 - s0
                tcap = int(tcap)
                for buf, vals in ((dstl_arr, dstl_s[s0:s1]),
                                  (w_arr, w_s[s0:s1]),
                                  (al_arr, al_full[src_s[s0:s1]]),
                                  (ar_arr, ar_full[dst_s[s0:s1]])):
                    b = np.zeros(tcap * 128, np.float32)
                    b[:nv] = vals
                    buf[:, col:col + tcap] = b.reshape(tcap, 128).T
                col += tcap
        blocks_k = np.array([np.where((block2core == k) & (block2slot == i))[0][0]
                             for i in range(BPC)])
        node0_k = node0_pad.reshape(NB, 128, D)[blocks_k].reshape(NSH, D)
        in_b.append({
            "aug": aug_full,
            "idx_lo": _pack_gather_idxs(lo_vals, t_lo, flags_lo),
            "idx_hi": _pack_gather_idxs(hi_vals, t_hi, flags_hi),
            "dstl": dstl_arr,
            "wgt": w_arr,
            "alv": al_arr,
            "arv": ar_arr,
            "node0_sh": node0_k,
            "gb": gb,
            "iota_in": iota_np,
        })
        _cache.setdefault("blocks_by_core", {})[k] = blocks_k
    res_b = run_bass_kernel_spmd(nc_b, in_b, list(range(NCORES)),
                                 **_cache.get("runkw", {}))
    out = np.empty((NB, 128, D), np.float32)
    for k in range(NCORES):
        out[_cache["blocks_by_core"][k]] = \
            res_b.results[k]["out_sh"].reshape(BPC, 128, D)
    out = out.reshape(NPAD, D)
    t_b = res_b.exec_time_ns
    _cache["t_a_ns"] = t_a
    _cache["t_b_ns"] = t_b
    if t_a is not None and t_b is not None:
        _cache["last_exec_ns"] = t_a + t_b
    return out[:N]



# revision 2
# speedup vs baseline: 1.0966x; 1.0966x over previous
"""FAConv + LayerNorm + ReLU fused Trainium2 kernel (8 NeuronCores, SPMD).

Strategy (v2):
  Host: sort edges by destination 128-node block (core k owns 49 blocks =
  a contiguous 6272-node output shard -> no all-reduce), split each block's
  edges by src < 25088 (int16 gather-index limit), pad per (block, half) to
  tiles of 128 edges.
  Phase A (data-parallel): whole-shard SBUF residency; a_l/a_r = node @ att
  via one broadcast DVE multiply + log2 tree reduction per vector; bf16 node
  table cast on Scalar engine; 3 big DMAs total.
  Host: concat shards; permute a_l by edge src and a_r by edge dst into the
  padded tile layout (data movement only - all arithmetic stays on device).
  Phase B (edge-parallel): coef = tanh(a_l[src]+a_r[dst])*w as whole-array
  ops; one-hot stat tiles built 32-tiles-at-a-time with stride-0 broadcast
  APs (2 DVE ops per 32 tiles); src rows dma_gathered in full-ring 1024-row
  calls that cross block boundaries; segment-sum as PSUM-accumulated matmuls
  with a 0.1*I identity matmul folding in the eps*node_0 skip; LayerNorm
  stats accumulated per block by 2 Scalar-engine ops during PSUM drain;
  normalization + ReLU applied whole-array at the end.
"""
import sys

for _p in ('/opt/trn_rl_repo', '/root/.axon_site/_ro/trn_rl_repo'):
    if _p not in sys.path:
        sys.path.insert(0, _p)

import numpy as np
import ml_dtypes

import concourse.bass as bass
import concourse.bacc as bacc
import concourse.tile as tile
from concourse import mybir
from concourse.bass_utils import run_bass_kernel_spmd

N = 50000
D = 256
NCORES = 8
BPC = 49                    # dst blocks per core
NPAD = NCORES * BPC * 128   # 50176
NSH = BPC * 128             # 6272 nodes per core shard
HALF = NPAD // 2            # 25088 (int16-safe gather index range)
EPS_FA = 0.1
EPS_LN = 1e-5
MAXG = 8                    # tiles (of 128 idxs) per dma_gather (ring cap 1024)
CSTAT = 32                  # tiles per batched one-hot build

f32 = mybir.dt.float32
bf16 = mybir.dt.bfloat16
i16 = mybir.dt.int16
AF = mybir.ActivationFunctionType
OP = mybir.AluOpType

_cache = {}


def _shard_ap(t):
    """DRAM AP over a [NSH, D] tensor iterated as [128 part, BPC, D]:
    partition p, block i, col c -> row i*128+p. Lets one DMA move the whole
    per-core shard to/from an SBUF [128, BPC, D] tile."""
    return bass.AP(t, 0, [(D, 128), (128 * D, BPC), (1, D)])


def _build_phase_a():
    nc = bacc.Bacc("TRN2", target_bir_lowering=False, debug=False,
                   num_devices=NCORES)
    node_sh = nc.declare_dram_parameter("node_sh", [NSH, D], f32, isOutput=False)
    att = nc.declare_dram_parameter("att", [2, D], f32, isOutput=False)
    aug_sh = nc.declare_dram_parameter("aug_sh", [NSH, D], bf16, isOutput=True)
    alr_sh = nc.declare_dram_parameter("alr_sh", [128, 2 * BPC], f32, isOutput=True)

    with tile.TileContext(nc) as tc:
        with (
            tc.tile_pool(name="const", bufs=1) as cpool,
            tc.tile_pool(name="big", bufs=1) as big,
            tc.tile_pool(name="psum", bufs=2, space="PSUM") as psum,
        ):
            ones = cpool.tile([1, 128], f32)
            nc.vector.memset(ones[:], 1.0)
            att_bc = []
            for j in range(2):
                att_row = cpool.tile([1, D], f32, tag=f"attrow{j}")
                nc.sync.dma_start(out=att_row[:], in_=att[j:j + 1, :])
                ps = psum.tile([128, D], f32, tag="attps")
                nc.tensor.matmul(out=ps[:], lhsT=ones[:], rhs=att_row[:],
                                 start=True, stop=True)
                bc = cpool.tile([128, 1, D], f32, tag=f"attbc{j}")
                nc.vector.tensor_copy(bc[:, 0, :], ps[:])
                att_bc.append(bc)

            # chunked whole-shard processing: load / cast / reduce per chunk
            # so DMA overlaps the DVE tree work
            NCHUNK = 4
            bounds = [round(BPC * i / NCHUNK) for i in range(NCHUNK + 1)]
            alr = big.tile([128, 2 * BPC], f32, tag="alr")
            for ci in range(NCHUNK):
                b0, b1 = bounds[ci], bounds[ci + 1]
                nb = b1 - b0
                node_c = big.tile([128, nb, D], f32, tag=f"node{ci}")
                nc.sync.dma_start(
                    out=node_c[:],
                    in_=bass.AP(node_sh, b0 * 128 * D,
                                [(D, 128), (128 * D, nb), (1, D)]))
                # bf16 cast on Scalar engine (overlaps the DVE tree below)
                aug_c = big.tile([128, nb, D], bf16, tag=f"aug{ci}")
                nc.scalar.activation(out=aug_c[:], in_=node_c[:], func=AF.Copy)
                nc.sync.dma_start(
                    out=bass.AP(aug_sh, b0 * 128 * D,
                                [(D, 128), (128 * D, nb), (1, D)]),
                    in_=aug_c[:])
                # a_l / a_r: broadcast multiply then log2 tree-sum
                for j in range(2):
                    scr = big.tile([128, nb, D], f32, tag=f"scr{ci}_{j}")
                    nc.vector.tensor_tensor(
                        out=scr[:], in0=node_c[:],
                        in1=att_bc[j][:].to_broadcast([128, nb, D]),
                        op=OP.mult)
                    w = D
                    while w > 1:
                        h = w // 2
                        nc.vector.tensor_tensor(
                            out=scr[:, :, 0:h], in0=scr[:, :, 0:h],
                            in1=scr[:, :, h:w], op=OP.add)
                        w = h
                    nc.vector.tensor_copy(
                        alr[:, j * BPC + b0:j * BPC + b1], scr[:, :, 0])
            nc.sync.dma_start(out=alr_sh[:, :], in_=alr[:])
    nc.finalize()
    return nc


def _build_phase_b(t_lo, t_hi, gb_identity):
    TT = int(sum(t_lo) + sum(t_hi))          # total edge tiles
    TL = int(sum(t_lo))
    TH = int(sum(t_hi))
    nc = bacc.Bacc("TRN2", target_bir_lowering=False, debug=False,
                   num_devices=NCORES, num_swdge_queues=4)
    aug = nc.declare_dram_parameter("aug", [NPAD, D], bf16, isOutput=False)
    idx_lo = nc.declare_dram_parameter("idx_lo", [128, max(8 * TL, 8)], i16,
                                       isOutput=False)
    idx_hi = nc.declare_dram_parameter("idx_hi", [128, max(8 * TH, 8)], i16,
                                       isOutput=False)
    dstl = nc.declare_dram_parameter("dstl", [128, TT], bf16, isOutput=False)
    wgt = nc.declare_dram_parameter("wgt", [128, TT], f32, isOutput=False)
    alv = nc.declare_dram_parameter("alv", [128, TT], f32, isOutput=False)
    arv = nc.declare_dram_parameter("arv", [128, TT], f32, isOutput=False)
    node0_sh = nc.declare_dram_parameter("node0_sh", [NSH, D], bf16,
                                         isOutput=False)
    gb = nc.declare_dram_parameter("gb", [1, 2 * D], f32, isOutput=False)
    iota_in = nc.declare_dram_parameter("iota_in", [128, 128], bf16,
                                        isOutput=False)
    epsi_in = nc.declare_dram_parameter("epsi_in", [128, 128], bf16,
                                        isOutput=False)
    out_sh = nc.declare_dram_parameter("out_sh", [NSH, D], f32, isOutput=True)

    # per-block tile order is lo tiles then hi tiles; gather calls are
    # full-ring chunks of the global lo / hi streams (cross block bounds)
    with tile.TileContext(nc) as tc:
        with (
            tc.tile_pool(name="const", bufs=1) as cpool,
            tc.tile_pool(name="big", bufs=1) as big,
            tc.tile_pool(name="glo", bufs=6) as glo,
            tc.tile_pool(name="ghi", bufs=6) as ghi,
            tc.tile_pool(name="eq", bufs=2) as eqp,
            tc.tile_pool(name="stat", bufs=2) as statp,
            tc.tile_pool(name="epi", bufs=2) as epi,
            tc.tile_pool(name="psum", bufs=4, space="PSUM") as psum,
            tc.tile_pool(name="gbps", bufs=1, space="PSUM") as gbpsum,
        ):
            # ---- constants ----
            iota_bf = cpool.tile([128, 1, 128], bf16)
            nc.sync.dma_start(out=iota_bf[:, 0, :], in_=iota_in[:, :])
            epsi = cpool.tile([128, 128], bf16)
            nc.sync.dma_start(out=epsi[:], in_=epsi_in[:, :])
            if not gb_identity:
                ones_f = cpool.tile([1, 128], f32)
                nc.vector.memset(ones_f[:], 1.0)
                gb_row = cpool.tile([1, 2 * D], f32)
                nc.sync.dma_start(out=gb_row[:], in_=gb[:, :])
                gb_ps = gbpsum.tile([128, 2 * D], f32, tag="gbps")
                nc.tensor.matmul(out=gb_ps[:], lhsT=ones_f[:], rhs=gb_row[:],
                                 start=True, stop=True)
                gb_bc = cpool.tile([128, 2 * D], f32)
                nc.vector.tensor_copy(gb_bc[:], gb_ps[:])

            # ---- stream preload ----
            ilo = cpool.tile([128, max(8 * TL, 8)], i16, tag="ilo")
            nc.sync.dma_start(out=ilo[:], in_=idx_lo[:, :])
            ihi = cpool.tile([128, max(8 * TH, 8)], i16, tag="ihi")
            nc.sync.dma_start(out=ihi[:], in_=idx_hi[:, :])
            dstl_sb = cpool.tile([128, TT], bf16, tag="dstl")
            nc.sync.dma_start(out=dstl_sb[:], in_=dstl[:, :])
            w_sb = cpool.tile([128, TT], f32, tag="w")
            nc.sync.dma_start(out=w_sb[:], in_=wgt[:, :])
            al_sb = cpool.tile([128, TT], f32, tag="al")
            nc.sync.dma_start(out=al_sb[:], in_=alv[:, :])
            ar_sb = cpool.tile([128, TT], f32, tag="ar")
            nc.sync.dma_start(out=ar_sb[:], in_=arv[:, :])
            # chunked node0 preload: first matmul only needs block 0
            node0_big = big.tile([128, BPC, D], bf16, tag="node0")
            n0_bounds = [0, 4, 16, BPC]
            for b0, b1 in zip(n0_bounds[:-1], n0_bounds[1:]):
                nc.sync.dma_start(
                    out=node0_big[:, b0:b1, :],
                    in_=bass.AP(node0_sh, b0 * 128 * D,
                                [(D, 128), (128 * D, b1 - b0), (1, D)]))

            # ---- whole-array coef = tanh(al + ar) * w, cast bf16 ----
            arg_sb = cpool.tile([128, TT], f32, tag="arg")
            nc.vector.tensor_tensor(out=arg_sb[:], in0=al_sb[:], in1=ar_sb[:],
                                    op=OP.add)
            nc.scalar.activation(out=arg_sb[:], in_=arg_sb[:], func=AF.Tanh)
            coef_sb = cpool.tile([128, TT], bf16, tag="coef")
            nc.vector.tensor_tensor(out=coef_sb[:], in0=arg_sb[:],
                                    in1=w_sb[:], op=OP.mult)

            # ---- LN stat accumulators + x staging ----
            x_big = big.tile([128, BPC, D], f32, tag="x")
            sumx = big.tile([128, BPC], f32, tag="sumx")
            sumsq = big.tile([128, BPC], f32, tag="sumsq")
            sq_scr = epi.tile([128, D], f32, tag="sqscr")

            # ---- main loop ----
            # gather stream state per half: (pool, idx sbuf, col cursor,
            # current gb tile, slot cursor, remaining tiles)
            # Tile assigns DMASW sems round-robin (mod 8) over Pool DMA
            # instructions in program order, and a sem must stay locked to
            # one SWDGE queue -> queue must be global_call_idx % 4.
            qctr = [0]

            class GS:
                def __init__(self, pool, isb, total):
                    self.pool, self.isb, self.total = pool, isb, total
                    self.col = 0
                    self.done = 0
                    self.gbt = None
                    self.slot = 0
                    self.cap = 0

            def next_tile(gs):
                if gs.gbt is None or gs.slot == gs.cap:
                    c = min(MAXG, gs.total - gs.done)
                    gs.gbt = gs.pool.tile([128, MAXG, D], bf16, tag="g")
                    nc.gpsimd.dma_gather(
                        out_ap=gs.gbt[:, 0:c, :], in_ap=gs.base,
                        idxs_ap=gs.isb[:, gs.col:gs.col + 8 * c],
                        num_idxs=c * 128, num_idxs_reg=c * 128,
                        elem_size=D,
                        queue_num=qctr[0] % 4)
                    qctr[0] += 1
                    gs.col += 8 * c
                    gs.done += c
                    gs.slot, gs.cap = 0, c
                t = gs.gbt[:, gs.slot, :]
                gs.slot += 1
                return t

            lo = GS(glo, ilo, TL)
            lo.base = aug[0:HALF, :]
            hi = GS(ghi, ihi, TH)
            hi.base = aug[HALF:NPAD, :]

            gt = 0              # global tile cursor (stat/stream column)
            stat_t = None       # current stat chunk tile
            stat_off = CSTAT    # slot within chunk
            ep_bounds = [0, 12, 24, 36, BPC]   # epilogue group boundaries
            for i in range(BPC):
                acc = psum.tile([128, D], f32, tag="acc")
                # eps * node_0 via 0.1*I identity matmul (starts the chain)
                nc.tensor.matmul(out=acc[:], lhsT=epsi[:],
                                 rhs=node0_big[:, i, :],
                                 start=True, stop=False)
                ti = int(t_lo[i] + t_hi[i])
                for k in range(ti):
                    if stat_off == CSTAT:
                        c = min(CSTAT, TT - gt)
                        eq = eqp.tile([128, CSTAT, 128], bf16, tag="eq")
                        nc.vector.tensor_tensor(
                            out=eq[:, 0:c, :],
                            in0=iota_bf[:].to_broadcast([128, c, 128]),
                            in1=dstl_sb[:, gt:gt + c].to_broadcast(
                                [128, c, 128]),
                            op=OP.is_equal)
                        stat_t = statp.tile([128, CSTAT, 128], bf16,
                                            tag="stat")
                        nc.vector.tensor_tensor(
                            out=stat_t[:, 0:c, :],
                            in0=eq[:, 0:c, :],
                            in1=coef_sb[:, gt:gt + c].to_broadcast(
                                [128, c, 128]),
                            op=OP.mult)
                        stat_off = 0
                    gs = lo if k < int(t_lo[i]) else hi
                    g = next_tile(gs)
                    nc.tensor.matmul(out=acc[:],
                                     lhsT=stat_t[:, stat_off, :],
                                     rhs=g,
                                     start=False, stop=(k == ti - 1))
                    stat_off += 1
                    gt += 1
                # drain PSUM: x = acc (already includes eps*node0);
                # accumulate LN sums on Scalar engine
                nc.scalar.activation(out=x_big[:, i, :], in_=acc[:],
                                     func=AF.Copy,
                                     accum_out=sumx[:, i:i + 1])
                nc.scalar.activation(out=sq_scr[:], in_=acc[:],
                                     func=AF.Square,
                                     accum_out=sumsq[:, i:i + 1])

                # group epilogue: LayerNorm + ReLU + output DMA for finished
                # blocks, overlapped with the next group's matmul stream
                if i + 1 in ep_bounds:
                    g0 = ep_bounds[ep_bounds.index(i + 1) - 1]
                    g1 = i + 1
                    ng = g1 - g0
                    negmean = epi.tile([128, ng], f32, tag="negmean")
                    nc.scalar.activation(out=negmean[:],
                                         in_=sumx[:, g0:g1], func=AF.Copy,
                                         scale=-1.0 / D)
                    msq = epi.tile([128, ng], f32, tag="msq")
                    nc.vector.tensor_tensor(out=msq[:], in0=negmean[:],
                                            in1=negmean[:], op=OP.mult)
                    var = epi.tile([128, ng], f32, tag="var")
                    nc.scalar.activation(out=var[:], in_=sumsq[:, g0:g1],
                                         func=AF.Copy,
                                         scale=1.0 / D, bias=EPS_LN)
                    nc.vector.tensor_tensor(out=var[:], in0=var[:],
                                            in1=msq[:], op=OP.subtract)
                    std = epi.tile([128, ng], f32, tag="std")
                    nc.scalar.activation(out=std[:], in_=var[:],
                                         func=AF.Sqrt)
                    rstd = epi.tile([128, ng], f32, tag="rstd")
                    nc.vector.reciprocal(rstd[:], std[:])
                    xg = x_big[:, g0:g1, :]
                    nc.vector.tensor_tensor(
                        out=xg, in0=xg,
                        in1=negmean[:].to_broadcast([128, ng, D]), op=OP.add)
                    nc.vector.tensor_tensor(
                        out=xg, in0=xg,
                        in1=rstd[:].to_broadcast([128, ng, D]), op=OP.mult)
                    if not gb_identity:
                        nc.vector.tensor_tensor(
                            out=xg, in0=xg,
                            in1=gb_bc[:, 0:D].to_broadcast([128, ng, D]),
                            op=OP.mult)
                        nc.vector.tensor_tensor(
                            out=xg, in0=xg,
                            in1=gb_bc[:, D:2 * D].to_broadcast([128, ng, D]),
                            op=OP.add)
                    nc.vector.tensor_scalar_max(out=xg, in0=xg, scalar1=0.0)
                    nc.sync.dma_start(
                        out=bass.AP(out_sh, g0 * 128 * D,
                                    [(D, 128), (128 * D, ng), (1, D)]),
                        in_=xg)
    nc.finalize()
    # Tile assigns DMASW sems round-robin (mod 8) over Pool DMA insts in
    # FINAL scheduled order (which may differ from emission order), and a
    # sem must stay locked to one SWDGE queue -> rewrite queue_num to
    # final_order_idx % 4 so sem i%8 always pairs with queue i%4.
    gi = 0
    for bb in nc.m.functions[0].blocks:
        for inst in bb.instructions:
            if type(inst).__name__ == 'InstDMAGatherAnt':
                inst.queue_num = gi % 4
                gi += 1
    return nc


def _pack_gather_idxs(vals, total_tiles):
    """vals: concatenated int16 idx array (len <= total_tiles*128, zero-pad).
    Pack per dma_gather call (chunks of <= MAXG tiles), 16-wrapped and
    replicated across the 8 Q7-core partition groups."""
    ncols = 8 * int(total_tiles)
    arr = np.zeros((16, max(ncols, 8)), np.int16)
    v = np.zeros(int(total_tiles) * 128, np.int16)
    v[:len(vals)] = vals
    col = 0
    done = 0
    while done < total_tiles:
        c = int(min(MAXG, total_tiles - done))
        chunk = v[done * 128:(done + c) * 128]
        arr[:, col:col + 8 * c] = chunk.reshape(8 * c, 16).T
        col += 8 * c
        done += c
    return np.tile(arr, (8, 1))


def _host_shard(src, dst, w):
    # ---- host sharding prep ----
    # load-balance: rank dst blocks by edge count; slot i of the 8 cores
    # holds the blocks ranked [8i, 8i+8) -> per-slot max ~= mean -> minimal
    # SPMD padding. Output rows are re-assembled per assignment at the end.
    blk = dst >> 7
    NB = NCORES * BPC
    bcnt = np.bincount(blk, minlength=NB)
    ranked = np.argsort(-bcnt, kind="stable")
    block2core = np.empty(NB, np.int64)
    block2slot = np.empty(NB, np.int64)
    for r, b in enumerate(ranked):
        block2core[b] = r % NCORES
        block2slot[b] = r // NCORES
    key = (block2core[blk] * BPC + block2slot[blk]) * 2 + (src >= HALF)
    order = np.argsort(key, kind="stable")
    src_s = src[order].astype(np.int32)
    dst_s = dst[order].astype(np.int32)
    dstl_s = (dst_s & 127).astype(np.float32)
    w_s = w[order]
    cnt = np.bincount(key[order], minlength=2 * NCORES * BPC)
    offs = np.concatenate([[0], np.cumsum(cnt)])
    cnt = cnt.reshape(NCORES, BPC, 2)
    t_lo = np.maximum(1, -(-cnt[:, :, 0].max(axis=0) // 128))   # [BPC]
    t_hi = np.maximum(1, -(-cnt[:, :, 1].max(axis=0) // 128))
    return (block2core, block2slot, offs, src_s, dst_s, dstl_s, w_s,
            t_lo, t_hi)


def _build_in_b(node_0, aug_full, al_full, ar_full,
                block2core, block2slot, offs, src_s, dst_s, dstl_s, w_s,
                t_lo, t_hi, ln_weight, ln_bias):
    NB = NCORES * BPC
    TT = int(t_lo.sum() + t_hi.sum())
    node0_pad = np.zeros((NPAD, D), np.float32)
    node0_pad[:N] = node_0
    gbv = np.concatenate([ln_weight, ln_bias])[None, :]
    iota_np = np.tile(np.arange(128, dtype=np.float32).astype(
        ml_dtypes.bfloat16)[None, :], (128, 1))
    epsi_np = (EPS_FA * np.eye(128, dtype=np.float32)).astype(
        ml_dtypes.bfloat16)
    in_b = []
    for k in range(NCORES):
        lo_vals, hi_vals = [], []
        dstl_arr = np.zeros((128, TT), np.float32)
        w_arr = np.zeros((128, TT), np.float32)
        al_arr = np.zeros((128, TT), np.float32)
        ar_arr = np.zeros((128, TT), np.float32)
        col = 0
        for i in range(BPC):
            for h, coll, tcap in ((0, lo_vals, t_lo[i]), (1, hi_vals, t_hi[i])):
                ki = (2 * (k * BPC + i)) + h
                s0, s1 = offs[ki], offs[ki + 1]
                nv = s1 - s0
                tcap = int(tcap)
                v = np.zeros(tcap * 128, np.int16)
                sv = src_s[s0:s1]
                v[:nv] = (sv - HALF if h else sv)
                coll.append(v)
                for buf, vals in ((dstl_arr, dstl_s[s0:s1]),
                                  (w_arr, w_s[s0:s1]),
                                  (al_arr, al_full[src_s[s0:s1]]),
                                  (ar_arr, ar_full[dst_s[s0:s1]])):
                    b = np.zeros(tcap * 128, np.float32)
                    b[:nv] = vals
                    buf[:, col:col + tcap] = b.reshape(tcap, 128).T
                col += tcap
        blocks_k = np.array([np.where((block2core == k) & (block2slot == i))[0][0]
                             for i in range(BPC)])
        node0_k = node0_pad.reshape(NB, 128, D)[blocks_k].reshape(NSH, D)
        in_b.append({
            "aug": aug_full,
            "idx_lo": _pack_gather_idxs(np.concatenate(lo_vals), t_lo.sum()),
            "idx_hi": _pack_gather_idxs(np.concatenate(hi_vals), t_hi.sum()),
            "dstl": dstl_arr.astype(ml_dtypes.bfloat16),
            "wgt": w_arr,
            "alv": al_arr,
            "arv": ar_arr,
            "node0_sh": node0_k.astype(ml_dtypes.bfloat16),
            "gb": gbv,
            "iota_in": iota_np,
            "epsi_in": epsi_np,
        })
        _cache.setdefault("blocks_by_core", {})[k] = blocks_k
    return in_b


def kernel(node, node_0, edge_index, edge_attr, batch_ptr,
           att_l, att_r, ln_weight, ln_bias):
    node = np.asarray(node, np.float32)
    node_0 = np.asarray(node_0, np.float32)
    src = np.asarray(edge_index[0], np.int64)
    dst = np.asarray(edge_index[1], np.int64)
    w = np.asarray(edge_attr, np.float32)
    att_l = np.asarray(att_l, np.float32)
    att_r = np.asarray(att_r, np.float32)
    ln_weight = np.asarray(ln_weight, np.float32)
    ln_bias = np.asarray(ln_bias, np.float32)

    (block2core, block2slot, offs, src_s, dst_s, dstl_s, w_s,
     t_lo, t_hi) = _host_shard(src, dst, w)

    gb_identity = bool(np.all(ln_weight == 1.0) and np.all(ln_bias == 0.0))
    sig = (tuple(t_lo), tuple(t_hi), gb_identity)
    if "A" not in _cache:
        _cache["A"] = _build_phase_a()
    if ("B", sig) not in _cache:
        _cache[("B", sig)] = _build_phase_b(t_lo, t_hi, sig[2])
    nc_a = _cache["A"]
    nc_b = _cache[("B", sig)]

    # ---- phase A ----
    node_pad = np.zeros((NPAD, D), np.float32)
    node_pad[:N] = node
    att = np.stack([att_l, att_r])
    in_a = [{"node_sh": node_pad[k * NSH:(k + 1) * NSH], "att": att}
            for k in range(NCORES)]
    res_a = run_bass_kernel_spmd(nc_a, in_a, list(range(NCORES)),
                                 **_cache.get("runkw", {}))
    aug_full = np.concatenate([res_a.results[k]["aug_sh"]
                               for k in range(NCORES)])
    al_full = np.concatenate(
        [res_a.results[k]["alr_sh"][:, 0:BPC].T.reshape(NSH)
         for k in range(NCORES)])
    ar_full = np.concatenate(
        [res_a.results[k]["alr_sh"][:, BPC:2 * BPC].T.reshape(NSH)
         for k in range(NCORES)])
    t_a = res_a.exec_time_ns

    # ---- phase B ----
    in_b = _build_in_b(node_0, aug_full, al_full, ar_full,
                       block2core, block2slot, offs, src_s, dst_s, dstl_s,
                       w_s, t_lo, t_hi, ln_weight, ln_bias)
    res_b = run_bass_kernel_spmd(nc_b, in_b, list(range(NCORES)),
                                 **_cache.get("runkw", {}))
    NB = NCORES * BPC
    out = np.empty((NB, 128, D), np.float32)
    for k in range(NCORES):
        out[_cache["blocks_by_core"][k]] = \
            res_b.results[k]["out_sh"].reshape(BPC, 128, D)
    out = out.reshape(NPAD, D)
    t_b = res_b.exec_time_ns
    _cache["t_a_ns"] = t_a
    _cache["t_b_ns"] = t_b
    if t_a is not None and t_b is not None:
        _cache["last_exec_ns"] = t_a + t_b
    return out[:N]


# revision 3
# speedup vs baseline: 1.1114x; 1.0135x over previous
"""FAConv + LayerNorm + ReLU fused Trainium2 kernel (8 NeuronCores, SPMD).

Strategy (v2):
  Host: sort edges by destination 128-node block (core k owns 49 blocks =
  a contiguous 6272-node output shard -> no all-reduce), split each block's
  edges by src < 25088 (int16 gather-index limit), pad per (block, half) to
  tiles of 128 edges.
  Phase A (data-parallel): whole-shard SBUF residency; a_l/a_r = node @ att
  via one broadcast DVE multiply + log2 tree reduction per vector; bf16 node
  table cast on Scalar engine; 3 big DMAs total.
  Host: concat shards; permute a_l by edge src and a_r by edge dst into the
  padded tile layout (data movement only - all arithmetic stays on device).
  Phase B (edge-parallel): coef = tanh(a_l[src]+a_r[dst])*w as whole-array
  ops; one-hot stat tiles built 32-tiles-at-a-time with stride-0 broadcast
  APs (2 DVE ops per 32 tiles); src rows dma_gathered in full-ring 1024-row
  calls that cross block boundaries; segment-sum as PSUM-accumulated matmuls
  with a 0.1*I identity matmul folding in the eps*node_0 skip; LayerNorm
  stats accumulated per block by 2 Scalar-engine ops during PSUM drain;
  normalization + ReLU applied whole-array at the end.
"""
import sys

for _p in ('/opt/trn_rl_repo', '/root/.axon_site/_ro/trn_rl_repo'):
    if _p not in sys.path:
        sys.path.insert(0, _p)

import numpy as np
import ml_dtypes

import concourse.bass as bass
import concourse.bacc as bacc
import concourse.tile as tile
from concourse import mybir
from concourse.bass_utils import run_bass_kernel_spmd

N = 50000
D = 256
NCORES = 8
BPC = 49                    # dst blocks per core
NPAD = NCORES * BPC * 128   # 50176
NSH = BPC * 128             # 6272 nodes per core shard
HALF = NPAD // 2            # 25088 (int16-safe gather index range)
EPS_FA = 0.1
EPS_LN = 1e-5
MAXG = 8                    # tiles (of 128 idxs) per dma_gather (ring cap 1024)
CSTAT = 32                  # tiles per batched one-hot build

f32 = mybir.dt.float32
bf16 = mybir.dt.bfloat16
i16 = mybir.dt.int16
AF = mybir.ActivationFunctionType
OP = mybir.AluOpType

_cache = {}


def _shard_ap(t):
    """DRAM AP over a [NSH, D] tensor iterated as [128 part, BPC, D]:
    partition p, block i, col c -> row i*128+p. Lets one DMA move the whole
    per-core shard to/from an SBUF [128, BPC, D] tile."""
    return bass.AP(t, 0, [(D, 128), (128 * D, BPC), (1, D)])


def _build_phase_a():
    nc = bacc.Bacc("TRN2", target_bir_lowering=False, debug=False,
                   num_devices=NCORES)
    node_sh = nc.declare_dram_parameter("node_sh", [NSH, D], f32, isOutput=False)
    att = nc.declare_dram_parameter("att", [2, D], f32, isOutput=False)
    aug_sh = nc.declare_dram_parameter("aug_sh", [NSH, D], bf16, isOutput=True)
    alr_sh = nc.declare_dram_parameter("alr_sh", [128, 2 * BPC], f32, isOutput=True)

    with tile.TileContext(nc) as tc:
        with (
            tc.tile_pool(name="const", bufs=1) as cpool,
            tc.tile_pool(name="big", bufs=1) as big,
            tc.tile_pool(name="psum", bufs=2, space="PSUM") as psum,
        ):
            ones = cpool.tile([1, 128], f32)
            nc.vector.memset(ones[:], 1.0)
            att_bc = []
            for j in range(2):
                att_row = cpool.tile([1, D], f32, tag=f"attrow{j}")
                nc.sync.dma_start(out=att_row[:], in_=att[j:j + 1, :])
                ps = psum.tile([128, D], f32, tag="attps")
                nc.tensor.matmul(out=ps[:], lhsT=ones[:], rhs=att_row[:],
                                 start=True, stop=True)
                bc = cpool.tile([128, 1, D], f32, tag=f"attbc{j}")
                nc.vector.tensor_copy(bc[:, 0, :], ps[:])
                att_bc.append(bc)

            # chunked whole-shard processing: load / cast / reduce per chunk
            # so DMA overlaps the DVE tree work
            NCHUNK = 4
            bounds = [round(BPC * i / NCHUNK) for i in range(NCHUNK + 1)]
            alr = big.tile([128, 2 * BPC], f32, tag="alr")
            for ci in range(NCHUNK):
                b0, b1 = bounds[ci], bounds[ci + 1]
                nb = b1 - b0
                node_c = big.tile([128, nb, D], f32, tag=f"node{ci}")
                nc.sync.dma_start(
                    out=node_c[:],
                    in_=bass.AP(node_sh, b0 * 128 * D,
                                [(D, 128), (128 * D, nb), (1, D)]))
                # bf16 cast on Scalar engine (overlaps the DVE tree below)
                aug_c = big.tile([128, nb, D], bf16, tag=f"aug{ci}")
                nc.scalar.activation(out=aug_c[:], in_=node_c[:], func=AF.Copy)
                nc.sync.dma_start(
                    out=bass.AP(aug_sh, b0 * 128 * D,
                                [(D, 128), (128 * D, nb), (1, D)]),
                    in_=aug_c[:])
                # a_l: broadcast multiply + log2 tree-sum on DVE;
                # a_r: broadcast multiply on DVE, per-block accum on Scalar
                # (splits the reduction work across both engines)
                scr = big.tile([128, nb, D], f32, tag=f"scr{ci}_0")
                nc.vector.tensor_tensor(
                    out=scr[:], in0=node_c[:],
                    in1=att_bc[0][:].to_broadcast([128, nb, D]),
                    op=OP.mult)
                nc.vector.tensor_reduce(
                    out=alr[:, b0:b1], in_=scr[:],
                    axis=mybir.AxisListType.X, op=OP.add)
                scr_r = big.tile([128, nb, D], f32, tag=f"scr{ci}_1")
                nc.vector.tensor_tensor(
                    out=scr_r[:], in0=node_c[:],
                    in1=att_bc[1][:].to_broadcast([128, nb, D]),
                    op=OP.mult)
                for bi in range(nb):
                    nc.scalar.activation(
                        out=scr_r[:, bi, :], in_=scr_r[:, bi, :],
                        func=AF.Copy,
                        accum_out=alr[:, BPC + b0 + bi:BPC + b0 + bi + 1])
            nc.sync.dma_start(out=alr_sh[:, :], in_=alr[:])
    nc.finalize()
    return nc


def _build_phase_b(t_lo, t_hi, gb_identity):
    TT = int(sum(t_lo) + sum(t_hi))          # total edge tiles
    TL = int(sum(t_lo))
    TH = int(sum(t_hi))
    nc = bacc.Bacc("TRN2", target_bir_lowering=False, debug=False,
                   num_devices=NCORES, num_swdge_queues=4)
    aug = nc.declare_dram_parameter("aug", [NPAD, D], bf16, isOutput=False)
    idx_lo = nc.declare_dram_parameter("idx_lo", [128, max(8 * TL, 8)], i16,
                                       isOutput=False)
    idx_hi = nc.declare_dram_parameter("idx_hi", [128, max(8 * TH, 8)], i16,
                                       isOutput=False)
    dstl = nc.declare_dram_parameter("dstl", [128, TT], bf16, isOutput=False)
    wgt = nc.declare_dram_parameter("wgt", [128, TT], f32, isOutput=False)
    alv = nc.declare_dram_parameter("alv", [128, TT], f32, isOutput=False)
    arv = nc.declare_dram_parameter("arv", [128, TT], f32, isOutput=False)
    node0_sh = nc.declare_dram_parameter("node0_sh", [NSH, D], bf16,
                                         isOutput=False)
    gb = nc.declare_dram_parameter("gb", [1, 2 * D], f32, isOutput=False)
    iota_in = nc.declare_dram_parameter("iota_in", [128, 128], bf16,
                                        isOutput=False)
    epsi_in = nc.declare_dram_parameter("epsi_in", [128, 128], bf16,
                                        isOutput=False)
    out_sh = nc.declare_dram_parameter("out_sh", [NSH, D], f32, isOutput=True)

    # per-block tile order is lo tiles then hi tiles; gather calls are
    # full-ring chunks of the global lo / hi streams (cross block bounds)
    with tile.TileContext(nc) as tc:
        with (
            tc.tile_pool(name="const", bufs=1) as cpool,
            tc.tile_pool(name="big", bufs=1) as big,
            tc.tile_pool(name="glo", bufs=6) as glo,
            tc.tile_pool(name="ghi", bufs=6) as ghi,
            tc.tile_pool(name="eq", bufs=2) as eqp,
            tc.tile_pool(name="stat", bufs=2) as statp,
            tc.tile_pool(name="epi", bufs=2) as epi,
            tc.tile_pool(name="psum", bufs=4, space="PSUM") as psum,
            tc.tile_pool(name="gbps", bufs=1, space="PSUM") as gbpsum,
        ):
            # ---- gather idx streams first: nothing else gates the gathers
            ilo = cpool.tile([128, max(8 * TL, 8)], i16, tag="ilo")
            nc.sync.dma_start(out=ilo[:], in_=idx_lo[:, :])
            ihi = cpool.tile([128, max(8 * TH, 8)], i16, tag="ihi")
            nc.sync.dma_start(out=ihi[:], in_=idx_hi[:, :])

            # ---- constants ----
            iota_bf = cpool.tile([128, 1, 128], bf16)
            nc.sync.dma_start(out=iota_bf[:, 0, :], in_=iota_in[:, :])
            epsi = cpool.tile([128, 128], bf16)
            nc.sync.dma_start(out=epsi[:], in_=epsi_in[:, :])
            if not gb_identity:
                ones_f = cpool.tile([1, 128], f32)
                nc.vector.memset(ones_f[:], 1.0)
                gb_row = cpool.tile([1, 2 * D], f32)
                nc.sync.dma_start(out=gb_row[:], in_=gb[:, :])
                gb_ps = gbpsum.tile([128, 2 * D], f32, tag="gbps")
                nc.tensor.matmul(out=gb_ps[:], lhsT=ones_f[:], rhs=gb_row[:],
                                 start=True, stop=True)
                gb_bc = cpool.tile([128, 2 * D], f32)
                nc.vector.tensor_copy(gb_bc[:], gb_ps[:])

            # ---- stream preload ----
            dstl_sb = cpool.tile([128, TT], bf16, tag="dstl")
            nc.sync.dma_start(out=dstl_sb[:], in_=dstl[:, :])
            w_sb = cpool.tile([128, TT], f32, tag="w")
            nc.sync.dma_start(out=w_sb[:], in_=wgt[:, :])
            al_sb = cpool.tile([128, TT], f32, tag="al")
            nc.sync.dma_start(out=al_sb[:], in_=alv[:, :])
            ar_sb = cpool.tile([128, TT], f32, tag="ar")
            nc.sync.dma_start(out=ar_sb[:], in_=arv[:, :])
            # chunked node0 preload: first matmul only needs block 0
            node0_big = big.tile([128, BPC, D], bf16, tag="node0")
            n0_bounds = [0, 4, 16, BPC]
            for b0, b1 in zip(n0_bounds[:-1], n0_bounds[1:]):
                nc.sync.dma_start(
                    out=node0_big[:, b0:b1, :],
                    in_=bass.AP(node0_sh, b0 * 128 * D,
                                [(D, 128), (128 * D, b1 - b0), (1, D)]))

            # ---- whole-array coef = tanh(al + ar) * w, cast bf16 ----
            arg_sb = cpool.tile([128, TT], f32, tag="arg")
            nc.vector.tensor_tensor(out=arg_sb[:], in0=al_sb[:], in1=ar_sb[:],
                                    op=OP.add)
            nc.scalar.activation(out=arg_sb[:], in_=arg_sb[:], func=AF.Tanh)
            coef_sb = cpool.tile([128, TT], bf16, tag="coef")
            nc.vector.tensor_tensor(out=coef_sb[:], in0=arg_sb[:],
                                    in1=w_sb[:], op=OP.mult)

            # ---- LN stat accumulators + x staging ----
            x_big = big.tile([128, BPC, D], f32, tag="x")
            sumx = big.tile([128, BPC], f32, tag="sumx")
            sumsq = big.tile([128, BPC], f32, tag="sumsq")
            sq_scr = epi.tile([128, D], f32, tag="sqscr")

            # ---- main loop ----
            # gather stream state per half: (pool, idx sbuf, col cursor,
            # current gb tile, slot cursor, remaining tiles)
            # Tile assigns DMASW sems round-robin (mod 8) over Pool DMA
            # instructions in program order, and a sem must stay locked to
            # one SWDGE queue -> queue must be global_call_idx % 4.
            qctr = [0]

            class GS:
                def __init__(self, pool, isb, total):
                    self.pool, self.isb, self.total = pool, isb, total
                    self.col = 0
                    self.done = 0
                    self.gbt = None
                    self.slot = 0
                    self.cap = 0

            def next_tile(gs):
                if gs.gbt is None or gs.slot == gs.cap:
                    c = min(MAXG, gs.total - gs.done)
                    gs.gbt = gs.pool.tile([128, MAXG, D], bf16, tag="g")
                    nc.gpsimd.dma_gather(
                        out_ap=gs.gbt[:, 0:c, :], in_ap=gs.base,
                        idxs_ap=gs.isb[:, gs.col:gs.col + 8 * c],
                        num_idxs=c * 128, num_idxs_reg=c * 128,
                        elem_size=D,
                        queue_num=qctr[0] % 4)
                    qctr[0] += 1
                    gs.col += 8 * c
                    gs.done += c
                    gs.slot, gs.cap = 0, c
                t = gs.gbt[:, gs.slot, :]
                gs.slot += 1
                return t

            lo = GS(glo, ilo, TL)
            lo.base = aug[0:HALF, :]
            hi = GS(ghi, ihi, TH)
            hi.base = aug[HALF:NPAD, :]

            gt = 0              # global tile cursor (stat/stream column)
            stat_t = None       # current stat chunk tile
            stat_off = CSTAT    # slot within chunk
            ep_bounds = [0, 12, 24, 34, 43, BPC]  # epilogue group boundaries
            for i in range(BPC):
                acc = psum.tile([128, D], f32, tag="acc")
                # eps * node_0 via 0.1*I identity matmul (starts the chain)
                nc.tensor.matmul(out=acc[:], lhsT=epsi[:],
                                 rhs=node0_big[:, i, :],
                                 start=True, stop=False)
                ti = int(t_lo[i] + t_hi[i])
                for k in range(ti):
                    if stat_off == CSTAT:
                        c = min(CSTAT, TT - gt)
                        eq = eqp.tile([128, CSTAT, 128], bf16, tag="eq")
                        nc.vector.tensor_tensor(
                            out=eq[:, 0:c, :],
                            in0=iota_bf[:].to_broadcast([128, c, 128]),
                            in1=dstl_sb[:, gt:gt + c].to_broadcast(
                                [128, c, 128]),
                            op=OP.is_equal)
                        stat_t = statp.tile([128, CSTAT, 128], bf16,
                                            tag="stat")
                        nc.vector.tensor_tensor(
                            out=stat_t[:, 0:c, :],
                            in0=eq[:, 0:c, :],
                            in1=coef_sb[:, gt:gt + c].to_broadcast(
                                [128, c, 128]),
                            op=OP.mult)
                        stat_off = 0
                    gs = lo if k < int(t_lo[i]) else hi
                    g = next_tile(gs)
                    nc.tensor.matmul(out=acc[:],
                                     lhsT=stat_t[:, stat_off, :],
                                     rhs=g,
                                     start=False, stop=(k == ti - 1))
                    stat_off += 1
                    gt += 1
                # drain PSUM: x = acc (already includes eps*node0);
                # accumulate LN sums on Scalar engine
                nc.scalar.activation(out=x_big[:, i, :], in_=acc[:],
                                     func=AF.Copy,
                                     accum_out=sumx[:, i:i + 1])
                nc.scalar.activation(out=sq_scr[:], in_=acc[:],
                                     func=AF.Square,
                                     accum_out=sumsq[:, i:i + 1])

                # group epilogue: LayerNorm + ReLU + output DMA for finished
                # blocks, overlapped with the next group's matmul stream
                if i + 1 in ep_bounds:
                    g0 = ep_bounds[ep_bounds.index(i + 1) - 1]
                    g1 = i + 1
                    ng = g1 - g0
                    negmean = epi.tile([128, ng], f32, tag="negmean")
                    nc.scalar.activation(out=negmean[:],
                                         in_=sumx[:, g0:g1], func=AF.Copy,
                                         scale=-1.0 / D)
                    msq = epi.tile([128, ng], f32, tag="msq")
                    nc.vector.tensor_tensor(out=msq[:], in0=negmean[:],
                                            in1=negmean[:], op=OP.mult)
                    var = epi.tile([128, ng], f32, tag="var")
                    nc.scalar.activation(out=var[:], in_=sumsq[:, g0:g1],
                                         func=AF.Copy,
                                         scale=1.0 / D, bias=EPS_LN)
                    nc.vector.tensor_tensor(out=var[:], in0=var[:],
                                            in1=msq[:], op=OP.subtract)
                    std = epi.tile([128, ng], f32, tag="std")
                    nc.scalar.activation(out=std[:], in_=var[:],
                                         func=AF.Sqrt)
                    rstd = epi.tile([128, ng], f32, tag="rstd")
                    nc.vector.reciprocal(rstd[:], std[:])
                    xg = x_big[:, g0:g1, :]
                    nc.vector.tensor_tensor(
                        out=xg, in0=xg,
                        in1=negmean[:].to_broadcast([128, ng, D]), op=OP.add)
                    nc.vector.tensor_tensor(
                        out=xg, in0=xg,
                        in1=rstd[:].to_broadcast([128, ng, D]), op=OP.mult)
                    if not gb_identity:
                        nc.vector.tensor_tensor(
                            out=xg, in0=xg,
                            in1=gb_bc[:, 0:D].to_broadcast([128, ng, D]),
                            op=OP.mult)
                        nc.vector.tensor_tensor(
                            out=xg, in0=xg,
                            in1=gb_bc[:, D:2 * D].to_broadcast([128, ng, D]),
                            op=OP.add)
                    nc.scalar.activation(out=xg, in_=xg, func=AF.Relu)
                    nc.sync.dma_start(
                        out=bass.AP(out_sh, g0 * 128 * D,
                                    [(D, 128), (128 * D, ng), (1, D)]),
                        in_=xg)
    nc.finalize()
    # Tile assigns DMASW sems round-robin (mod 8) over Pool DMA insts in
    # FINAL scheduled order (which may differ from emission order), and a
    # sem must stay locked to one SWDGE queue -> rewrite queue_num to
    # final_order_idx % 4 so sem i%8 always pairs with queue i%4.
    gi = 0
    for bb in nc.m.functions[0].blocks:
        for inst in bb.instructions:
            if type(inst).__name__ == 'InstDMAGatherAnt':
                inst.queue_num = gi % 4
                gi += 1
    return nc


def _pack_gather_idxs(vals, total_tiles):
    """vals: concatenated int16 idx array (len <= total_tiles*128, zero-pad).
    Pack per dma_gather call (chunks of <= MAXG tiles), 16-wrapped and
    replicated across the 8 Q7-core partition groups."""
    ncols = 8 * int(total_tiles)
    arr = np.zeros((16, max(ncols, 8)), np.int16)
    v = np.zeros(int(total_tiles) * 128, np.int16)
    v[:len(vals)] = vals
    col = 0
    done = 0
    while done < total_tiles:
        c = int(min(MAXG, total_tiles - done))
        chunk = v[done * 128:(done + c) * 128]
        arr[:, col:col + 8 * c] = chunk.reshape(8 * c, 16).T
        col += 8 * c
        done += c
    return np.tile(arr, (8, 1))


def _host_shard(src, dst, w):
    # ---- host sharding prep ----
    # load-balance: rank dst blocks by edge count; slot i of the 8 cores
    # holds the blocks ranked [8i, 8i+8) -> per-slot max ~= mean -> minimal
    # SPMD padding. Output rows are re-assembled per assignment at the end.
    blk = dst >> 7
    NB = NCORES * BPC
    bcnt = np.bincount(blk, minlength=NB)
    ranked = np.argsort(-bcnt, kind="stable")
    block2core = np.empty(NB, np.int64)
    block2slot = np.empty(NB, np.int64)
    for r, b in enumerate(ranked):
        block2core[b] = r % NCORES
        block2slot[b] = r // NCORES
    lo_cnt = np.bincount(blk[src < HALF], minlength=NB)
    hi_cnt = np.bincount(blk[src >= HALF], minlength=NB)
    block2slot = _rebalance_slots(block2core, block2slot, lo_cnt, hi_cnt)
    key = (block2core[blk] * BPC + block2slot[blk]) * 2 + (src >= HALF)
    order = np.argsort(key, kind="stable")
    src_s = src[order].astype(np.int32)
    dst_s = dst[order].astype(np.int32)
    dstl_s = (dst_s & 127).astype(np.float32)
    w_s = w[order]
    cnt = np.bincount(key[order], minlength=2 * NCORES * BPC)
    offs = np.concatenate([[0], np.cumsum(cnt)])
    cnt = cnt.reshape(NCORES, BPC, 2)
    t_lo = np.maximum(1, -(-cnt[:, :, 0].max(axis=0) // 128))   # [BPC]
    t_hi = np.maximum(1, -(-cnt[:, :, 1].max(axis=0) // 128))
    return (block2core, block2slot, offs, src_s, dst_s, dstl_s, w_s,
            t_lo, t_hi)


def _rebalance_slots(block2core, block2slot, lo_cnt, hi_cnt):
    """Local search: swap slot assignments of block pairs within a core to
    shrink sum_i max_k ceil(cnt/128) (the SPMD tile padding)."""
    NB = NCORES * BPC
    L = np.zeros((NCORES, BPC), np.int64)
    H = np.zeros((NCORES, BPC), np.int64)
    blocks = np.zeros((NCORES, BPC), np.int64)
    for b in range(NB):
        k, s = block2core[b], block2slot[b]
        L[k, s] = lo_cnt[b]
        H[k, s] = hi_cnt[b]
        blocks[k, s] = b

    def slot_cost(s):
        return (max(1, -(-L[:, s].max() // 128))
                + max(1, -(-H[:, s].max() // 128)))

    cost = np.array([slot_cost(s) for s in range(BPC)])
    rng = np.random.default_rng(7)
    ks = rng.integers(0, NCORES, 60000)
    sa_ = rng.integers(0, BPC, 60000)
    sb_ = rng.integers(0, BPC, 60000)
    for k, sa, sb in zip(ks, sa_, sb_):
        if sa == sb:
            continue
        old = cost[sa] + cost[sb]
        L[k, sa], L[k, sb] = L[k, sb], L[k, sa]
        H[k, sa], H[k, sb] = H[k, sb], H[k, sa]
        ca, cb = slot_cost(sa), slot_cost(sb)
        if ca + cb <= old:
            cost[sa], cost[sb] = ca, cb
            blocks[k, sa], blocks[k, sb] = blocks[k, sb], blocks[k, sa]
        else:
            L[k, sa], L[k, sb] = L[k, sb], L[k, sa]
            H[k, sa], H[k, sb] = H[k, sb], H[k, sa]
    b2s = np.empty(NB, np.int64)
    for k in range(NCORES):
        for s in range(BPC):
            b2s[blocks[k, s]] = s
    return b2s


def _build_in_b(node_0, aug_full, al_full, ar_full,
                block2core, block2slot, offs, src_s, dst_s, dstl_s, w_s,
                t_lo, t_hi, ln_weight, ln_bias):
    NB = NCORES * BPC
    TT = int(t_lo.sum() + t_hi.sum())
    node0_pad = np.zeros((NPAD, D), np.float32)
    node0_pad[:N] = node_0
    gbv = np.concatenate([ln_weight, ln_bias])[None, :]
    iota_np = np.tile(np.arange(128, dtype=np.float32).astype(
        ml_dtypes.bfloat16)[None, :], (128, 1))
    epsi_np = (EPS_FA * np.eye(128, dtype=np.float32)).astype(
        ml_dtypes.bfloat16)
    in_b = []
    for k in range(NCORES):
        lo_vals, hi_vals = [], []
        dstl_arr = np.zeros((128, TT), np.float32)
        w_arr = np.zeros((128, TT), np.float32)
        al_arr = np.zeros((128, TT), np.float32)
        ar_arr = np.zeros((128, TT), np.float32)
        col = 0
        for i in range(BPC):
            for h, coll, tcap in ((0, lo_vals, t_lo[i]), (1, hi_vals, t_hi[i])):
                ki = (2 * (k * BPC + i)) + h
                s0, s1 = offs[ki], offs[ki + 1]
                nv = s1 - s0
                tcap = int(tcap)
                # pad slots gather scattered rows (not row 0: repeated
                # same-row reads serialize on one HBM channel); coef=0
                # zeroes their contribution.
                v = (np.arange(tcap * 128, dtype=np.int64) * 97
                     % HALF).astype(np.int16)
                sv = src_s[s0:s1]
                v[:nv] = (sv - HALF if h else sv)
                coll.append(v)
                for buf, vals in ((dstl_arr, dstl_s[s0:s1]),
                                  (w_arr, w_s[s0:s1]),
                                  (al_arr, al_full[src_s[s0:s1]]),
                                  (ar_arr, ar_full[dst_s[s0:s1]])):
                    b = np.zeros(tcap * 128, np.float32)
                    b[:nv] = vals
                    buf[:, col:col + tcap] = b.reshape(tcap, 128).T
                col += tcap
        blocks_k = np.array([np.where((block2core == k) & (block2slot == i))[0][0]
                             for i in range(BPC)])
        node0_k = node0_pad.reshape(NB, 128, D)[blocks_k].reshape(NSH, D)
        in_b.append({
            "aug": aug_full,
            "idx_lo": _pack_gather_idxs(np.concatenate(lo_vals), t_lo.sum()),
            "idx_hi": _pack_gather_idxs(np.concatenate(hi_vals), t_hi.sum()),
            "dstl": dstl_arr.astype(ml_dtypes.bfloat16),
            "wgt": w_arr,
            "alv": al_arr,
            "arv": ar_arr,
            "node0_sh": node0_k.astype(ml_dtypes.bfloat16),
            "gb": gbv,
            "iota_in": iota_np,
            "epsi_in": epsi_np,
        })
        _cache.setdefault("blocks_by_core", {})[k] = blocks_k
    return in_b


def kernel(node, node_0, edge_index, edge_attr, batch_ptr,
           att_l, att_r, ln_weight, ln_bias):
    node = np.asarray(node, np.float32)
    node_0 = np.asarray(node_0, np.float32)
    src = np.asarray(edge_index[0], np.int64)
    dst = np.asarray(edge_index[1], np.int64)
    w = np.asarray(edge_attr, np.float32)
    att_l = np.asarray(att_l, np.float32)
    att_r = np.asarray(att_r, np.float32)
    ln_weight = np.asarray(ln_weight, np.float32)
    ln_bias = np.asarray(ln_bias, np.float32)

    (block2core, block2slot, offs, src_s, dst_s, dstl_s, w_s,
     t_lo, t_hi) = _host_shard(src, dst, w)

    gb_identity = bool(np.all(ln_weight == 1.0) and np.all(ln_bias == 0.0))
    sig = (tuple(t_lo), tuple(t_hi), gb_identity)
    if "A" not in _cache:
        _cache["A"] = _build_phase_a()
    if ("B", sig) not in _cache:
        _cache[("B", sig)] = _build_phase_b(t_lo, t_hi, sig[2])
    nc_a = _cache["A"]
    nc_b = _cache[("B", sig)]

    # ---- phase A ----
    node_pad = np.zeros((NPAD, D), np.float32)
    node_pad[:N] = node
    att = np.stack([att_l, att_r])
    in_a = [{"node_sh": node_pad[k * NSH:(k + 1) * NSH], "att": att}
            for k in range(NCORES)]
    res_a = run_bass_kernel_spmd(nc_a, in_a, list(range(NCORES)),
                                 **_cache.get("runkw", {}))
    aug_full = np.concatenate([res_a.results[k]["aug_sh"]
                               for k in range(NCORES)])
    al_full = np.concatenate(
        [res_a.results[k]["alr_sh"][:, 0:BPC].T.reshape(NSH)
         for k in range(NCORES)])
    ar_full = np.concatenate(
        [res_a.results[k]["alr_sh"][:, BPC:2 * BPC].T.reshape(NSH)
         for k in range(NCORES)])
    t_a = res_a.exec_time_ns

    # ---- phase B ----
    in_b = _build_in_b(node_0, aug_full, al_full, ar_full,
                       block2core, block2slot, offs, src_s, dst_s, dstl_s,
                       w_s, t_lo, t_hi, ln_weight, ln_bias)
    res_b = run_bass_kernel_spmd(nc_b, in_b, list(range(NCORES)),
                                 **_cache.get("runkw", {}))
    NB = NCORES * BPC
    out = np.empty((NB, 128, D), np.float32)
    for k in range(NCORES):
        out[_cache["blocks_by_core"][k]] = \
            res_b.results[k]["out_sh"].reshape(BPC, 128, D)
    out = out.reshape(NPAD, D)
    t_b = res_b.exec_time_ns
    _cache["t_a_ns"] = t_a
    _cache["t_b_ns"] = t_b
    if t_a is not None and t_b is not None:
        _cache["last_exec_ns"] = t_a + t_b
    return out[:N]


# revision 4
# speedup vs baseline: 1.1685x; 1.0513x over previous
"""FAConv + LayerNorm + ReLU fused Trainium2 kernel (8 NeuronCores, SPMD).

Strategy (v2):
  Host: sort edges by destination 128-node block (core k owns 49 blocks =
  a contiguous 6272-node output shard -> no all-reduce), split each block's
  edges by src < 25088 (int16 gather-index limit), pad per (block, half) to
  tiles of 128 edges.
  Phase A (data-parallel): whole-shard SBUF residency; a_l/a_r = node @ att
  via one broadcast DVE multiply + log2 tree reduction per vector; bf16 node
  table cast on Scalar engine; 3 big DMAs total.
  Host: concat shards; permute a_l by edge src and a_r by edge dst into the
  padded tile layout (data movement only - all arithmetic stays on device).
  Phase B (edge-parallel): coef = tanh(a_l[src]+a_r[dst])*w as whole-array
  ops; one-hot stat tiles built 32-tiles-at-a-time with stride-0 broadcast
  APs (2 DVE ops per 32 tiles); src rows dma_gathered in full-ring 1024-row
  calls that cross block boundaries; segment-sum as PSUM-accumulated matmuls
  with a 0.1*I identity matmul folding in the eps*node_0 skip; LayerNorm
  stats accumulated per block by 2 Scalar-engine ops during PSUM drain;
  normalization + ReLU applied whole-array at the end.
"""
import sys

for _p in ('/opt/trn_rl_repo', '/root/.axon_site/_ro/trn_rl_repo'):
    if _p not in sys.path:
        sys.path.insert(0, _p)

import numpy as np
import ml_dtypes

import concourse.bass as bass
import concourse.bacc as bacc
import concourse.tile as tile
from concourse import mybir
from concourse.bass_utils import run_bass_kernel_spmd

N = 50000
D = 256
NCORES = 8
BPC = 49                    # dst blocks per core
NPAD = NCORES * BPC * 128   # 50176
NSH = BPC * 128             # 6272 nodes per core shard
HALF = NPAD // 2            # 25088 (int16-safe gather index range)
EPS_FA = 0.1
EPS_LN = 1e-5
MAXG = 8                    # tiles (of 128 idxs) per dma_gather (ring cap 1024)
CSTAT = 32                  # tiles per batched one-hot build

f32 = mybir.dt.float32
bf16 = mybir.dt.bfloat16
i16 = mybir.dt.int16
AF = mybir.ActivationFunctionType
OP = mybir.AluOpType

_cache = {}


def _shard_ap(t):
    """DRAM AP over a [NSH, D] tensor iterated as [128 part, BPC, D]:
    partition p, block i, col c -> row i*128+p. Lets one DMA move the whole
    per-core shard to/from an SBUF [128, BPC, D] tile."""
    return bass.AP(t, 0, [(D, 128), (128 * D, BPC), (1, D)])


def _build_phase_a():
    nc = bacc.Bacc("TRN2", target_bir_lowering=False, debug=False,
                   num_devices=NCORES)
    node_sh = nc.declare_dram_parameter("node_sh", [NSH, D], f32, isOutput=False)
    att = nc.declare_dram_parameter("att", [2, D], f32, isOutput=False)
    aug_sh = nc.declare_dram_parameter("aug_sh", [NSH, D], bf16, isOutput=True)
    alr_sh = nc.declare_dram_parameter("alr_sh", [128, 2 * BPC], f32, isOutput=True)

    with tile.TileContext(nc) as tc:
        with (
            tc.tile_pool(name="const", bufs=1) as cpool,
            tc.tile_pool(name="big", bufs=1) as big,
            tc.tile_pool(name="psum", bufs=2, space="PSUM") as psum,
        ):
            ones = cpool.tile([1, 128], f32)
            nc.vector.memset(ones[:], 1.0)
            att_bc = []
            for j in range(2):
                att_row = cpool.tile([1, D], f32, tag=f"attrow{j}")
                nc.sync.dma_start(out=att_row[:], in_=att[j:j + 1, :])
                ps = psum.tile([128, D], f32, tag="attps")
                nc.tensor.matmul(out=ps[:], lhsT=ones[:], rhs=att_row[:],
                                 start=True, stop=True)
                bc = cpool.tile([128, 1, D], f32, tag=f"attbc{j}")
                nc.vector.tensor_copy(bc[:, 0, :], ps[:])
                att_bc.append(bc)

            # chunked whole-shard processing: load / cast / reduce per chunk
            # so DMA overlaps the DVE tree work
            NCHUNK = 4
            bounds = [round(BPC * i / NCHUNK) for i in range(NCHUNK + 1)]
            alr = big.tile([128, 2 * BPC], f32, tag="alr")
            for ci in range(NCHUNK):
                b0, b1 = bounds[ci], bounds[ci + 1]
                nb = b1 - b0
                node_c = big.tile([128, nb, D], f32, tag=f"node{ci}")
                nc.sync.dma_start(
                    out=node_c[:],
                    in_=bass.AP(node_sh, b0 * 128 * D,
                                [(D, 128), (128 * D, nb), (1, D)]))
                # bf16 cast on Scalar engine (overlaps the DVE tree below)
                aug_c = big.tile([128, nb, D], bf16, tag=f"aug{ci}")
                nc.scalar.activation(out=aug_c[:], in_=node_c[:], func=AF.Copy)
                nc.sync.dma_start(
                    out=bass.AP(aug_sh, b0 * 128 * D,
                                [(D, 128), (128 * D, nb), (1, D)]),
                    in_=aug_c[:])
                # a_l: broadcast multiply + log2 tree-sum on DVE;
                # a_r: broadcast multiply on DVE, per-block accum on Scalar
                # (splits the reduction work across both engines)
                scr = big.tile([128, nb, D], f32, tag=f"scr{ci}_0")
                nc.vector.tensor_tensor(
                    out=scr[:], in0=node_c[:],
                    in1=att_bc[0][:].to_broadcast([128, nb, D]),
                    op=OP.mult)
                nc.vector.tensor_reduce(
                    out=alr[:, b0:b1], in_=scr[:],
                    axis=mybir.AxisListType.X, op=OP.add)
                scr_r = big.tile([128, nb, D], f32, tag=f"scr{ci}_1")
                nc.gpsimd.tensor_tensor(
                    out=scr_r[:], in0=node_c[:],
                    in1=att_bc[1][:].to_broadcast([128, nb, D]),
                    op=OP.mult)
                for bi in range(nb):
                    nc.scalar.activation(
                        out=scr_r[:, bi, :], in_=scr_r[:, bi, :],
                        func=AF.Copy,
                        accum_out=alr[:, BPC + b0 + bi:BPC + b0 + bi + 1])
            nc.sync.dma_start(out=alr_sh[:, :], in_=alr[:])
    nc.finalize()
    return nc


def _build_phase_b(t_lo, t_hi, gb_identity):
    TT = int(sum(t_lo) + sum(t_hi))          # total edge tiles
    TL = int(sum(t_lo))
    TH = int(sum(t_hi))
    nc = bacc.Bacc("TRN2", target_bir_lowering=False, debug=False,
                   num_devices=NCORES, num_swdge_queues=4)
    aug = nc.declare_dram_parameter("aug", [NPAD, D], bf16, isOutput=False)
    idx_lo = nc.declare_dram_parameter("idx_lo", [128, max(8 * TL, 8)], i16,
                                       isOutput=False)
    idx_hi = nc.declare_dram_parameter("idx_hi", [128, max(8 * TH, 8)], i16,
                                       isOutput=False)
    dstl = nc.declare_dram_parameter("dstl", [128, TT], bf16, isOutput=False)
    wgt = nc.declare_dram_parameter("wgt", [128, TT], f32, isOutput=False)
    alv = nc.declare_dram_parameter("alv", [128, TT], f32, isOutput=False)
    arv = nc.declare_dram_parameter("arv", [128, TT], f32, isOutput=False)
    node0_sh = nc.declare_dram_parameter("node0_sh", [NSH, D], bf16,
                                         isOutput=False)
    gb = nc.declare_dram_parameter("gb", [1, 2 * D], f32, isOutput=False)
    iota_in = nc.declare_dram_parameter("iota_in", [128, 128], bf16,
                                        isOutput=False)
    epsi_in = nc.declare_dram_parameter("epsi_in", [128, 128], bf16,
                                        isOutput=False)
    out_sh = nc.declare_dram_parameter("out_sh", [NSH, D], f32, isOutput=True)

    # per-block tile order is lo tiles then hi tiles; gather calls are
    # full-ring chunks of the global lo / hi streams (cross block bounds)
    with tile.TileContext(nc) as tc:
        with (
            tc.tile_pool(name="const", bufs=1) as cpool,
            tc.tile_pool(name="big", bufs=1) as big,
            tc.tile_pool(name="glo", bufs=6) as glo,
            tc.tile_pool(name="ghi", bufs=6) as ghi,
            tc.tile_pool(name="eq", bufs=2) as eqp,
            tc.tile_pool(name="stat", bufs=2) as statp,
            tc.tile_pool(name="epi", bufs=2) as epi,
            tc.tile_pool(name="psum", bufs=4, space="PSUM") as psum,
            tc.tile_pool(name="gbps", bufs=1, space="PSUM") as gbpsum,
        ):
            # ---- gather idx streams first: nothing else gates the gathers
            ilo = cpool.tile([128, max(8 * TL, 8)], i16, tag="ilo")
            nc.sync.dma_start(out=ilo[:], in_=idx_lo[:, :])
            ihi = cpool.tile([128, max(8 * TH, 8)], i16, tag="ihi")
            nc.sync.dma_start(out=ihi[:], in_=idx_hi[:, :])

            # ---- constants ----
            iota_bf = cpool.tile([128, 1, 128], bf16)
            nc.sync.dma_start(out=iota_bf[:, 0, :], in_=iota_in[:, :])
            epsi = cpool.tile([128, 128], bf16)
            nc.sync.dma_start(out=epsi[:], in_=epsi_in[:, :])
            if not gb_identity:
                ones_f = cpool.tile([1, 128], f32)
                nc.vector.memset(ones_f[:], 1.0)
                gb_row = cpool.tile([1, 2 * D], f32)
                nc.sync.dma_start(out=gb_row[:], in_=gb[:, :])
                gb_ps = gbpsum.tile([128, 2 * D], f32, tag="gbps")
                nc.tensor.matmul(out=gb_ps[:], lhsT=ones_f[:], rhs=gb_row[:],
                                 start=True, stop=True)
                gb_bc = cpool.tile([128, 2 * D], f32)
                nc.vector.tensor_copy(gb_bc[:], gb_ps[:])

            # ---- stream preload ----
            dstl_sb = cpool.tile([128, TT], bf16, tag="dstl")
            nc.sync.dma_start(out=dstl_sb[:], in_=dstl[:, :])
            w_sb = cpool.tile([128, TT], f32, tag="w")
            nc.sync.dma_start(out=w_sb[:], in_=wgt[:, :])
            al_sb = cpool.tile([128, TT], f32, tag="al")
            nc.sync.dma_start(out=al_sb[:], in_=alv[:, :])
            ar_sb = cpool.tile([128, TT], f32, tag="ar")
            nc.sync.dma_start(out=ar_sb[:], in_=arv[:, :])
            # chunked node0 preload: first matmul only needs block 0
            node0_big = big.tile([128, BPC, D], bf16, tag="node0")
            n0_bounds = [0, 4, 16, BPC]
            for b0, b1 in zip(n0_bounds[:-1], n0_bounds[1:]):
                nc.sync.dma_start(
                    out=node0_big[:, b0:b1, :],
                    in_=bass.AP(node0_sh, b0 * 128 * D,
                                [(D, 128), (128 * D, b1 - b0), (1, D)]))

            # ---- whole-array coef = tanh(al + ar) * w, cast bf16 ----
            arg_sb = cpool.tile([128, TT], f32, tag="arg")
            nc.vector.tensor_tensor(out=arg_sb[:], in0=al_sb[:], in1=ar_sb[:],
                                    op=OP.add)
            nc.scalar.activation(out=arg_sb[:], in_=arg_sb[:], func=AF.Tanh)
            coef_sb = cpool.tile([128, TT], bf16, tag="coef")
            nc.vector.tensor_tensor(out=coef_sb[:], in0=arg_sb[:],
                                    in1=w_sb[:], op=OP.mult)

            # ---- LN stat accumulators + x staging ----
            x_big = big.tile([128, BPC, D], f32, tag="x")
            sumx = big.tile([128, BPC], f32, tag="sumx")
            sumsq = big.tile([128, BPC], f32, tag="sumsq")
            sq_scr = epi.tile([128, D], f32, tag="sqscr")

            # ---- main loop ----
            # gather stream state per half: (pool, idx sbuf, col cursor,
            # current gb tile, slot cursor, remaining tiles)
            # Tile assigns DMASW sems round-robin (mod 8) over Pool DMA
            # instructions in program order, and a sem must stay locked to
            # one SWDGE queue -> queue must be global_call_idx % 4.
            qctr = [0]

            class GS:
                def __init__(self, pool, isb, total):
                    self.pool, self.isb, self.total = pool, isb, total
                    self.col = 0
                    self.done = 0
                    self.gbt = None
                    self.slot = 0
                    self.cap = 0

            def next_tile(gs):
                if gs.gbt is None or gs.slot == gs.cap:
                    c = min(MAXG, gs.total - gs.done)
                    gs.gbt = gs.pool.tile([128, MAXG, D], bf16, tag="g")
                    nc.gpsimd.dma_gather(
                        out_ap=gs.gbt[:, 0:c, :], in_ap=gs.base,
                        idxs_ap=gs.isb[:, gs.col:gs.col + 8 * c],
                        num_idxs=c * 128, num_idxs_reg=c * 128,
                        elem_size=D,
                        queue_num=qctr[0] % 4)
                    qctr[0] += 1
                    gs.col += 8 * c
                    gs.done += c
                    gs.slot, gs.cap = 0, c
                t = gs.gbt[:, gs.slot, :]
                gs.slot += 1
                return t

            lo = GS(glo, ilo, TL)
            lo.base = aug[0:HALF, :]
            hi = GS(ghi, ihi, TH)
            hi.base = aug[HALF:NPAD, :]

            gt = 0              # global tile cursor (stat/stream column)
            stat_t = None       # current stat chunk tile
            stat_off = CSTAT    # slot within chunk
            ep_bounds = [0, 12, 24, 34, 43, BPC]  # epilogue group boundaries
            for i in range(BPC):
                acc = psum.tile([128, D], f32, tag="acc")
                # eps * node_0 via 0.1*I identity matmul (starts the chain)
                nc.tensor.matmul(out=acc[:], lhsT=epsi[:],
                                 rhs=node0_big[:, i, :],
                                 start=True, stop=False)
                ti = int(t_lo[i] + t_hi[i])
                for k in range(ti):
                    if stat_off == CSTAT:
                        c = min(CSTAT, TT - gt)
                        eq = eqp.tile([128, CSTAT, 128], bf16, tag="eq")
                        nc.vector.tensor_tensor(
                            out=eq[:, 0:c, :],
                            in0=iota_bf[:].to_broadcast([128, c, 128]),
                            in1=dstl_sb[:, gt:gt + c].to_broadcast(
                                [128, c, 128]),
                            op=OP.is_equal)
                        stat_t = statp.tile([128, CSTAT, 128], bf16,
                                            tag="stat")
                        nc.vector.tensor_tensor(
                            out=stat_t[:, 0:c, :],
                            in0=eq[:, 0:c, :],
                            in1=coef_sb[:, gt:gt + c].to_broadcast(
                                [128, c, 128]),
                            op=OP.mult)
                        stat_off = 0
                    gs = lo if k < int(t_lo[i]) else hi
                    g = next_tile(gs)
                    nc.tensor.matmul(out=acc[:],
                                     lhsT=stat_t[:, stat_off, :],
                                     rhs=g,
                                     start=False, stop=(k == ti - 1))
                    stat_off += 1
                    gt += 1
                # drain PSUM: x = acc (already includes eps*node0);
                # accumulate LN sums on Scalar engine
                nc.scalar.activation(out=x_big[:, i, :], in_=acc[:],
                                     func=AF.Copy,
                                     accum_out=sumx[:, i:i + 1])
                nc.scalar.activation(out=sq_scr[:], in_=acc[:],
                                     func=AF.Square,
                                     accum_out=sumsq[:, i:i + 1])

                # group epilogue: LayerNorm + ReLU + output DMA for finished
                # blocks, overlapped with the next group's matmul stream
                if i + 1 in ep_bounds:
                    g0 = ep_bounds[ep_bounds.index(i + 1) - 1]
                    g1 = i + 1
                    ng = g1 - g0
                    negmean = epi.tile([128, ng], f32, tag="negmean")
                    nc.scalar.activation(out=negmean[:],
                                         in_=sumx[:, g0:g1], func=AF.Copy,
                                         scale=-1.0 / D)
                    msq = epi.tile([128, ng], f32, tag="msq")
                    nc.vector.tensor_tensor(out=msq[:], in0=negmean[:],
                                            in1=negmean[:], op=OP.mult)
                    var = epi.tile([128, ng], f32, tag="var")
                    nc.scalar.activation(out=var[:], in_=sumsq[:, g0:g1],
                                         func=AF.Copy,
                                         scale=1.0 / D, bias=EPS_LN)
                    nc.vector.tensor_tensor(out=var[:], in0=var[:],
                                            in1=msq[:], op=OP.subtract)
                    std = epi.tile([128, ng], f32, tag="std")
                    nc.scalar.activation(out=std[:], in_=var[:],
                                         func=AF.Sqrt)
                    rstd = epi.tile([128, ng], f32, tag="rstd")
                    nc.vector.reciprocal(rstd[:], std[:])
                    nmr = epi.tile([128, ng], f32, tag="nmr")
                    nc.vector.tensor_tensor(out=nmr[:], in0=negmean[:],
                                            in1=rstd[:], op=OP.mult)
                    xg = x_big[:, g0:g1, :]
                    if gb_identity:
                        # whole LN+ReLU per block in ONE Act op:
                        # Relu(x*rstd + negmean*rstd), scale/bias are
                        # per-partition columns
                        for bi in range(ng):
                            nc.scalar.activation(
                                out=x_big[:, g0 + bi, :],
                                in_=x_big[:, g0 + bi, :],
                                func=AF.Relu,
                                scale=rstd[:, bi:bi + 1],
                                bias=nmr[:, bi:bi + 1])
                    else:
                        nc.vector.tensor_tensor(
                            out=xg, in0=xg,
                            in1=negmean[:].to_broadcast([128, ng, D]),
                            op=OP.add)
                        nc.vector.tensor_tensor(
                            out=xg, in0=xg,
                            in1=rstd[:].to_broadcast([128, ng, D]),
                            op=OP.mult)
                        nc.vector.tensor_tensor(
                            out=xg, in0=xg,
                            in1=gb_bc[:, 0:D].to_broadcast([128, ng, D]),
                            op=OP.mult)
                        nc.vector.tensor_tensor(
                            out=xg, in0=xg,
                            in1=gb_bc[:, D:2 * D].to_broadcast([128, ng, D]),
                            op=OP.add)
                        nc.scalar.activation(out=xg, in_=xg, func=AF.Relu)
                    nc.sync.dma_start(
                        out=bass.AP(out_sh, g0 * 128 * D,
                                    [(D, 128), (128 * D, ng), (1, D)]),
                        in_=xg)
    nc.finalize()
    # Tile assigns DMASW sems round-robin (mod 8) over Pool DMA insts in
    # FINAL scheduled order (which may differ from emission order), and a
    # sem must stay locked to one SWDGE queue -> rewrite queue_num to
    # final_order_idx % 4 so sem i%8 always pairs with queue i%4.
    gi = 0
    for bb in nc.m.functions[0].blocks:
        for inst in bb.instructions:
            if type(inst).__name__ == 'InstDMAGatherAnt':
                inst.queue_num = gi % 4
                gi += 1
    return nc


def _pack_gather_idxs(vals, total_tiles):
    """vals: concatenated int16 idx array (len <= total_tiles*128, zero-pad).
    Pack per dma_gather call (chunks of <= MAXG tiles), 16-wrapped and
    replicated across the 8 Q7-core partition groups."""
    ncols = 8 * int(total_tiles)
    arr = np.zeros((16, max(ncols, 8)), np.int16)
    v = np.zeros(int(total_tiles) * 128, np.int16)
    v[:len(vals)] = vals
    col = 0
    done = 0
    while done < total_tiles:
        c = int(min(MAXG, total_tiles - done))
        chunk = v[done * 128:(done + c) * 128]
        arr[:, col:col + 8 * c] = chunk.reshape(8 * c, 16).T
        col += 8 * c
        done += c
    return np.tile(arr, (8, 1))


def _host_shard(src, dst, w):
    # ---- host sharding prep ----
    # load-balance: rank dst blocks by edge count; slot i of the 8 cores
    # holds the blocks ranked [8i, 8i+8) -> per-slot max ~= mean -> minimal
    # SPMD padding. Output rows are re-assembled per assignment at the end.
    blk = dst >> 7
    NB = NCORES * BPC
    bcnt = np.bincount(blk, minlength=NB)
    ranked = np.argsort(-bcnt, kind="stable")
    block2core = np.empty(NB, np.int64)
    block2slot = np.empty(NB, np.int64)
    for r, b in enumerate(ranked):
        block2core[b] = r % NCORES
        block2slot[b] = r // NCORES
    lo_cnt = np.bincount(blk[src < HALF], minlength=NB)
    hi_cnt = np.bincount(blk[src >= HALF], minlength=NB)
    block2slot = _rebalance_slots(block2core, block2slot, lo_cnt, hi_cnt)
    key = (block2core[blk] * BPC + block2slot[blk]) * 2 + (src >= HALF)
    order = np.argsort(key, kind="stable")
    src_s = src[order].astype(np.int32)
    dst_s = dst[order].astype(np.int32)
    dstl_s = (dst_s & 127).astype(np.float32)
    w_s = w[order]
    cnt = np.bincount(key[order], minlength=2 * NCORES * BPC)
    offs = np.concatenate([[0], np.cumsum(cnt)])
    cnt = cnt.reshape(NCORES, BPC, 2)
    t_lo = np.maximum(1, -(-cnt[:, :, 0].max(axis=0) // 128))   # [BPC]
    t_hi = np.maximum(1, -(-cnt[:, :, 1].max(axis=0) // 128))
    return (block2core, block2slot, offs, src_s, dst_s, dstl_s, w_s,
            t_lo, t_hi)


def _rebalance_slots(block2core, block2slot, lo_cnt, hi_cnt):
    """Local search: swap slot assignments of block pairs within a core to
    shrink sum_i max_k ceil(cnt/128) (the SPMD tile padding)."""
    NB = NCORES * BPC
    L = np.zeros((NCORES, BPC), np.int64)
    H = np.zeros((NCORES, BPC), np.int64)
    blocks = np.zeros((NCORES, BPC), np.int64)
    for b in range(NB):
        k, s = block2core[b], block2slot[b]
        L[k, s] = lo_cnt[b]
        H[k, s] = hi_cnt[b]
        blocks[k, s] = b

    def slot_cost(s):
        return (max(1, -(-L[:, s].max() // 128))
                + max(1, -(-H[:, s].max() // 128)))

    cost = np.array([slot_cost(s) for s in range(BPC)])
    rng = np.random.default_rng(7)
    ks = rng.integers(0, NCORES, 60000)
    sa_ = rng.integers(0, BPC, 60000)
    sb_ = rng.integers(0, BPC, 60000)
    for k, sa, sb in zip(ks, sa_, sb_):
        if sa == sb:
            continue
        old = cost[sa] + cost[sb]
        L[k, sa], L[k, sb] = L[k, sb], L[k, sa]
        H[k, sa], H[k, sb] = H[k, sb], H[k, sa]
        ca, cb = slot_cost(sa), slot_cost(sb)
        if ca + cb <= old:
            cost[sa], cost[sb] = ca, cb
            blocks[k, sa], blocks[k, sb] = blocks[k, sb], blocks[k, sa]
        else:
            L[k, sa], L[k, sb] = L[k, sb], L[k, sa]
            H[k, sa], H[k, sb] = H[k, sb], H[k, sa]
    b2s = np.empty(NB, np.int64)
    for k in range(NCORES):
        for s in range(BPC):
            b2s[blocks[k, s]] = s
    return b2s


def _build_in_b(node_0, aug_full, al_full, ar_full,
                block2core, block2slot, offs, src_s, dst_s, dstl_s, w_s,
                t_lo, t_hi, ln_weight, ln_bias):
    NB = NCORES * BPC
    TT = int(t_lo.sum() + t_hi.sum())
    node0_pad = np.zeros((NPAD, D), np.float32)
    node0_pad[:N] = node_0
    gbv = np.concatenate([ln_weight, ln_bias])[None, :]
    iota_np = np.tile(np.arange(128, dtype=np.float32).astype(
        ml_dtypes.bfloat16)[None, :], (128, 1))
    epsi_np = (EPS_FA * np.eye(128, dtype=np.float32)).astype(
        ml_dtypes.bfloat16)
    in_b = []
    for k in range(NCORES):
        lo_vals, hi_vals = [], []
        dstl_arr = np.zeros((128, TT), np.float32)
        w_arr = np.zeros((128, TT), np.float32)
        al_arr = np.zeros((128, TT), np.float32)
        ar_arr = np.zeros((128, TT), np.float32)
        col = 0
        for i in range(BPC):
            for h, coll, tcap in ((0, lo_vals, t_lo[i]), (1, hi_vals, t_hi[i])):
                ki = (2 * (k * BPC + i)) + h
                s0, s1 = offs[ki], offs[ki + 1]
                nv = s1 - s0
                tcap = int(tcap)
                # pad slots gather scattered rows (not row 0: repeated
                # same-row reads serialize on one HBM channel); coef=0
                # zeroes their contribution.
                v = (np.arange(tcap * 128, dtype=np.int64) * 97
                     % HALF).astype(np.int16)
                sv = src_s[s0:s1]
                v[:nv] = (sv - HALF if h else sv)
                coll.append(v)
                for buf, vals in ((dstl_arr, dstl_s[s0:s1]),
                                  (w_arr, w_s[s0:s1]),
                                  (al_arr, al_full[src_s[s0:s1]]),
                                  (ar_arr, ar_full[dst_s[s0:s1]])):
                    b = np.zeros(tcap * 128, np.float32)
                    b[:nv] = vals
                    buf[:, col:col + tcap] = b.reshape(tcap, 128).T
                col += tcap
        blocks_k = np.array([np.where((block2core == k) & (block2slot == i))[0][0]
                             for i in range(BPC)])
        node0_k = node0_pad.reshape(NB, 128, D)[blocks_k].reshape(NSH, D)
        in_b.append({
            "aug": aug_full,
            "idx_lo": _pack_gather_idxs(np.concatenate(lo_vals), t_lo.sum()),
            "idx_hi": _pack_gather_idxs(np.concatenate(hi_vals), t_hi.sum()),
            "dstl": dstl_arr.astype(ml_dtypes.bfloat16),
            "wgt": w_arr,
            "alv": al_arr,
            "arv": ar_arr,
            "node0_sh": node0_k.astype(ml_dtypes.bfloat16),
            "gb": gbv,
            "iota_in": iota_np,
            "epsi_in": epsi_np,
        })
        _cache.setdefault("blocks_by_core", {})[k] = blocks_k
    return in_b


def kernel(node, node_0, edge_index, edge_attr, batch_ptr,
           att_l, att_r, ln_weight, ln_bias):
    node = np.asarray(node, np.float32)
    node_0 = np.asarray(node_0, np.float32)
    src = np.asarray(edge_index[0], np.int64)
    dst = np.asarray(edge_index[1], np.int64)
    w = np.asarray(edge_attr, np.float32)
    att_l = np.asarray(att_l, np.float32)
    att_r = np.asarray(att_r, np.float32)
    ln_weight = np.asarray(ln_weight, np.float32)
    ln_bias = np.asarray(ln_bias, np.float32)

    (block2core, block2slot, offs, src_s, dst_s, dstl_s, w_s,
     t_lo, t_hi) = _host_shard(src, dst, w)

    gb_identity = bool(np.all(ln_weight == 1.0) and np.all(ln_bias == 0.0))
    sig = (tuple(t_lo), tuple(t_hi), gb_identity)
    if "A" not in _cache:
        _cache["A"] = _build_phase_a()
    if ("B", sig) not in _cache:
        _cache[("B", sig)] = _build_phase_b(t_lo, t_hi, sig[2])
    nc_a = _cache["A"]
    nc_b = _cache[("B", sig)]

    # ---- phase A ----
    node_pad = np.zeros((NPAD, D), np.float32)
    node_pad[:N] = node
    att = np.stack([att_l, att_r])
    in_a = [{"node_sh": node_pad[k * NSH:(k + 1) * NSH], "att": att}
            for k in range(NCORES)]
    res_a = run_bass_kernel_spmd(nc_a, in_a, list(range(NCORES)),
                                 **_cache.get("runkw", {}))
    aug_full = np.concatenate([res_a.results[k]["aug_sh"]
                               for k in range(NCORES)])
    al_full = np.concatenate(
        [res_a.results[k]["alr_sh"][:, 0:BPC].T.reshape(NSH)
         for k in range(NCORES)])
    ar_full = np.concatenate(
        [res_a.results[k]["alr_sh"][:, BPC:2 * BPC].T.reshape(NSH)
         for k in range(NCORES)])
    t_a = res_a.exec_time_ns

    # ---- phase B ----
    in_b = _build_in_b(node_0, aug_full, al_full, ar_full,
                       block2core, block2slot, offs, src_s, dst_s, dstl_s,
                       w_s, t_lo, t_hi, ln_weight, ln_bias)
    res_b = run_bass_kernel_spmd(nc_b, in_b, list(range(NCORES)),
                                 **_cache.get("runkw", {}))
    NB = NCORES * BPC
    out = np.empty((NB, 128, D), np.float32)
    for k in range(NCORES):
        out[_cache["blocks_by_core"][k]] = \
            res_b.results[k]["out_sh"].reshape(BPC, 128, D)
    out = out.reshape(NPAD, D)
    t_b = res_b.exec_time_ns
    _cache["t_a_ns"] = t_a
    _cache["t_b_ns"] = t_b
    if t_a is not None and t_b is not None:
        _cache["last_exec_ns"] = t_a + t_b
    return out[:N]
